# revision 14
# baseline (speedup 1.0000x reference)
"""Trainium2 Bass kernel for nn_DecoderLayer_11424613007924.

DecoderLayer: block-sparse attention (BLEN=256, causal first block,
2D-local windowed tail blocks) + LayerNorm + MLP, fp32 I/O.

Sharding: data-parallel over batch. 32 samples -> 8 NeuronCores x 4.

Per-core dataflow (per sample):
  X [1024,256] --(cast bf16, DMA-transpose)--> X.T
  Q.T/K.T = W.T @ X.T (PE, bf16);  V = X.T-stationary @ Wv (token-major)
  per (block b, query-chunk qc, head h):
      S  = q.T-chunk.T @ k.T-window (+ mask via identity-matmul inject, PSUM)
      P  = exp(S)  (ACT, accum_out -> softmax denominators)
      P.T via DMA-transpose; O_u = P.T.T @ v (PE); O = O_u * recip(den) (DVE)
  y = O.T-stationary @ Wo; z1 = y + X (fp32)
  LN1 via bn_stats; z1n = (z1-m)*rstd (g1 folded into W1 host-side)
  h = z1n.T @ W1eff + b1eff (ones-row inject); r = relu(h)
  z2 = r.T @ W2 + (ln1_b+b2) inject + z1n*g1 inject (diag(g1) matmul)
  out = (z2-m2)*g2b*rstd2 + b2gb  (scalar_tensor_tensor x2)

Numerics: matmul operands bf16 (PSUM fp32 accum); residual stream,
LN stats, softmax denominators fp32. Masks use -30 instead of -1e9
(exp(-30)*512 ~ 5e-11 relative contamination).
"""
import numpy as np
import ml_dtypes

import concourse.bass as bass
import concourse.mybir as mybir
import concourse.tile as tile
from bass_rust import ScopedClock

BF = mybir.dt.bfloat16
F32 = mybir.dt.float32
AF = mybir.ActivationFunctionType
OP = mybir.AluOpType

N_CORES = 8
B, L, D = 32, 1024, 256
H, DH, F = 8, 32, 512
BLEN = 256
SPC = B // N_CORES  # samples per core
NB = L // BLEN      # 4 blocks
NEG = -30.0
EPS = 1e-6

# ---------------------------------------------------------------- fixups ---
# This container's walrus build rejects instructions carrying >1 semaphore
# wait. Split extra waits onto same-engine NOPs after Tile scheduling.


def _split_sync_waits(nc):
    for fn in nc.m.functions:
        for bb in fn.blocks:
            insts = bb.instructions
            if not insts:
                continue
            new_list = []
            n_split = 0
            for inst in insts:
                si = inst.sync_info
                waits = list(si.on_wait) if (si and si.on_wait) else []
                if len(waits) > 1:
                    si.on_wait = waits[:1]
                    for w in waits[1:]:
                        nop = nc.engines[inst.engine].nop()
                        for f2 in nc.m.functions:
                            for b2 in f2.blocks:
                                l2 = b2.instructions
                                if l2 and l2[-1] is nop.ins:
                                    l2.pop()
                        nop.ins.sync_info = mybir.SyncInfo(on_wait=[w], on_update=[])
                        new_list.append(nop.ins)
                        n_split += 1
                new_list.append(inst)
            if n_split:
                bb.instructions = new_list


def _patched_drain_and_barrier(self, tick_clock, wait_clock):
    nc = self.nc
    probe = nc.sync.nop()
    wait_clock.add_sem_waits(probe.ins, ScopedClock({None: tick_clock.global_clock}))
    nc.sync.drain()
    nc.all_engine_barrier()
    assert self.sems is not None
    popped = nc._tile_sem_poison_stack.pop()
    assert popped is self._sem_poison
    nc.clear_and_free_semaphores(list(self.sems.allocated().values()))
    nc.all_engine_barrier()


tile.TileContext._drain_and_barrier = _patched_drain_and_barrier


# ------------------------------------------------------------- host prep ---

def _mask2d(blen=BLEN, h=32, win=6):
    cp = np.arange(blen, 2 * blen)[:, None]
    op = np.arange(2 * blen)[None, :]
    causal = op <= cp
    ch, cw = cp // h, cp % h
    oh, ow = op // h, op % h
    ok = causal & (np.abs(ch - oh) <= win) & (np.abs(cw - ow) <= win)
    return np.where(ok, 0.0, NEG).astype(np.float32)


def _causal_mask(blen=BLEN):
    return np.where(np.tril(np.ones((blen, blen), bool)), 0.0, NEG).astype(np.float32)


def _bcast(ap, offset_extra, plist):
    """AP with explicit [step,count] free dims appended to partition dim."""
    return bass.AP(tensor=ap.tensor, offset=ap.offset + offset_extra,
                   ap=[list(ap.ap[0])] + plist)


def _mkap(ap, offset_extra, dims):
    """AP with fully explicit [step,count] dims (incl. partition dim)."""
    return bass.AP(tensor=ap.tensor, offset=ap.offset + offset_extra, ap=dims)


def _win(b, qc):
    """Key window (absolute token range) for block b, query chunk qc."""
    if b == 0:
        return 0, 128 * (qc + 1)
    a = (b - 1) * 256 + 64 + 128 * qc
    return a, a + 320




_CACHE = {}

# tuning knobs (read at build time)
KNOBS = {"psS": 3, "psO": 2, "psM": 3, "big": 2, "small": 2}


def _build():
    if "nc" in _CACHE:
        return _CACHE["nc"]
    nc = bass.Bass(target_bir_lowering=False)

    xin = nc.declare_dram_parameter("X", [SPC, L, D], F32, isOutput=False)
    out = nc.declare_dram_parameter("OUT", [SPC, L, D], F32, isOutput=True)
    wq = nc.declare_dram_parameter("wq", [D, D], BF, isOutput=False)
    wk = nc.declare_dram_parameter("wk", [D, D], BF, isOutput=False)
    wv = nc.declare_dram_parameter("wv", [D, D], BF, isOutput=False)
    wo = nc.declare_dram_parameter("wo", [D, D], BF, isOutput=False)
    w1 = nc.declare_dram_parameter("w1", [D, F], BF, isOutput=False)
    w2 = nc.declare_dram_parameter("w2", [F, D], BF, isOutput=False)
    dg1 = nc.declare_dram_parameter("dg1", [2, 128, D], BF, isOutput=False)
    b1col = nc.declare_dram_parameter("b1col", [128, 4], F32, isOutput=False)
    browz = nc.declare_dram_parameter("browz", [1, D], BF, isOutput=False)
    g2b = nc.declare_dram_parameter("g2b", [128, D], F32, isOutput=False)
    b2gb = nc.declare_dram_parameter("b2gb", [128, D], F32, isOutput=False)
    i128 = nc.declare_dram_parameter("i128", [128, 128], BF, isOutput=False)
    ones1 = nc.declare_dram_parameter("ones1", [1, 128], BF, isOutput=False)
    m01 = nc.declare_dram_parameter("m01", [2, 128, 320], BF, isOutput=False)
    mb0 = nc.declare_dram_parameter("mb0", [128, 128], BF, isOutput=False)
    mb1 = nc.declare_dram_parameter("mb1", [128, 256], BF, isOutput=False)

    with tile.TileContext(nc) as tc:
        with (
            tc.tile_pool(name="static", bufs=1) as st,
            tc.tile_pool(name="big", bufs=KNOBS["big"]) as bigp,
            tc.tile_pool(name="small", bufs=KNOBS["small"]) as smp,
            tc.tile_pool(name="psS", bufs=KNOBS["psS"], space="PSUM") as psS,
            tc.tile_pool(name="psO", bufs=KNOBS["psO"], space="PSUM") as psO,
            tc.tile_pool(name="psM", bufs=KNOBS["psM"], space="PSUM") as psM,
        ):
            # ---- statics
            # weights stored K-chunked: chunk kc lives at cols [kc*N, (kc+1)*N)
            s_wq = st.tile([128, 2 * D], BF, name="wq", tag="wq")
            s_wk = st.tile([128, 2 * D], BF, name="wk", tag="wk")
            s_wv = st.tile([128, 2 * D], BF, name="wv", tag="wv")
            s_wo = st.tile([128, 2 * D], BF, name="wo", tag="wo")
            s_w1 = st.tile([128, 2 * F], BF, name="w1", tag="w1")
            s_w2 = st.tile([128, 4 * D], BF, name="w2", tag="w2")
            s_dg1 = [st.tile([128, D], BF, name=f"dg1{c}", tag=f"dg1{c}") for c in range(2)]
            s_b1col = st.tile([128, 4], F32, name="b1col", tag="b1col")
            s_browz = st.tile([1, D], BF, name="browz", tag="browz")
            s_g2b = st.tile([128, D], F32, name="g2b", tag="g2b")
            s_b2gb = st.tile([128, D], F32, name="b2gb", tag="b2gb")
            s_i128 = st.tile([128, 128], BF, name="i128", tag="i128")
            s_ones1 = st.tile([1, 128], BF, name="ones1", tag="ones1")
            s_m01 = [st.tile([128, 320], BF, name=f"m01_{qc}", tag=f"m01_{qc}") for qc in range(2)]
            s_mb0 = st.tile([128, 128], BF, name="mb0", tag="mb0")
            s_eps = st.tile([128, 1], F32, name="eps", tag="eps")
            s_mb1 = st.tile([128, 256], BF, name="mb1", tag="mb1")
            # qk-projection weights first: sample 0's first matmuls need them
            for dst, dsrc, nch_, w_ in [
                (s_wq, wq, 2, D), (s_wk, wk, 2, D), (s_wv, wv, 2, D),
                (s_wo, wo, 2, D), (s_w1, w1, 2, F), (s_w2, w2, 4, D),
            ]:
                for kc in range(nch_):
                    nc.sync.dma_start(out=dst[:, kc * w_:(kc + 1) * w_],
                                      in_=dsrc[kc * 128:(kc + 1) * 128, :])
            for dst, dsrc in [
                (s_mb0, mb0), (s_mb1, mb1), (s_i128, i128), (s_ones1, ones1),
                (s_b1col, b1col), (s_browz, browz),
                (s_g2b, g2b), (s_b2gb, b2gb),
            ]:
                nc.sync.dma_start(out=dst[:], in_=dsrc[:])
            nc.vector.memset(s_eps[:], EPS)
            for c in range(2):
                nc.sync.dma_start(out=s_dg1[c][:], in_=dg1[c])
                nc.sync.dma_start(out=s_m01[c][:], in_=m01[c])

            # weight chunk kc (K-rows kc*128..) cols [mlo,mhi) of a w_-wide chunk
            def wch(t, kc, w_, mlo, mhi):
                return t[:, kc * w_ + mlo:kc * w_ + mhi]

            # ---- X load / cast / transpose, software-pipelined -------------
            # Emitted one sample ahead (prefetch) on the scalar+vector queues
            # so the chain never sits behind the current sample's sync-queue
            # transposes (in-order queues = head-of-line blocking).
            xt_c, xT_c = {}, {}

            def prefetch(s):
                xt_all = bigp.tile([128, 8 * D], F32, name="xt_all", tag="xt_all")
                nc.scalar.dma_start(
                    out=xt_all[:],
                    in_=_mkap(xin[s, 0:1, 0:1], 0,
                              [[D, 128], [128 * D, 8], [1, D]]))
                # xb_all col = dc*1024 + tc*128 + p' (C-major, C = dc*8+tc)
                # so ONE batched transpose yields xTall col = C*128 + j
                #   = dc*1024 + tc*128 + j  (the layout consumers expect)
                xb_all = bigp.tile([128, 8 * D], BF, name="xb_all", tag="xb_all")
                nc.vector.tensor_copy(
                    _bcast(xb_all[0:128, 0:1], 0,
                           [[128, 8], [1024, 2], [1, 128]]),
                    _bcast(xt_all[0:128, 0:1], 0,
                           [[256, 8], [128, 2], [1, 128]]))
                xTall = bigp.tile([128, 2 * L], BF, name="xTall", tag="xTall")
                nc.scalar.dma_start_transpose(
                    out=_bcast(xTall[0:128, 0:1], 0, [[128, 16], [1, 128]]),
                    in_=xb_all[:])
                xt_c[s] = [xt_all[:, tc_ * D:(tc_ + 1) * D] for tc_ in range(8)]
                xT_c[s] = xTall

            prefetch(0)
            for s in range(SPC):
                if s + 1 < SPC:
                    prefetch(s + 1)
                xt, xTall = xt_c.pop(s), xT_c.pop(s)

                # ---- Q.T / K.T (d-major) and V (token-major) ---------------
                qT = [bigp.tile([128, L], BF, name=f"qT{mc}", tag=f"qT{mc}") for mc in range(2)]
                kT = [bigp.tile([128, L], BF, name=f"kT{mc}", tag=f"kT{mc}") for mc in range(2)]
                for wt, dstl in ((s_wq, qT), (s_wk, kT)):
                    for mc in range(2):
                        for hf in range(2):
                            ps = psM.tile([128, 512], F32, name="mm", tag="mm")
                            for kc in range(2):
                                nc.tensor.matmul(
                                    ps[:], lhsT=wch(wt, kc, D, mc * 128, mc * 128 + 128),
                                    rhs=xTall[:, kc * L + hf * 512:kc * L + (hf + 1) * 512],
                                    start=(kc == 0), stop=(kc == 1))
                            nc.vector.tensor_copy(
                                dstl[mc][:, hf * 512:(hf + 1) * 512], ps[:])
                # head h%4==3 sits at base partition 96 (invalid for matmul
                # operands); extract to offset-0 tiles via SBUF->SBUF DMA
                q3 = [smp.tile([32, L], BF, name=f"q3_{hc}", tag=f"q3_{hc}") for hc in range(2)]
                k3 = [smp.tile([32, L], BF, name=f"k3_{hc}", tag=f"k3_{hc}") for hc in range(2)]
                for hc in range(2):
                    nc.sync.dma_start(out=q3[hc][:], in_=qT[hc][96:128, :])
                    nc.sync.dma_start(out=k3[hc][:], in_=kT[hc][96:128, :])
                # vb: 0-aligned V token-chunks (block0 needs tokens [0,256)).
                # vb64: 64-shifted chunks, vb64[t] rows <-> tokens 64+t*128..,
                # aligning tail-window AV chunks to a single matmul each.
                # Layout [128, 8*33]: head h at cols h*33..h*33+32, col h*33+32
                # is ones -> AV matmul emits softmax denominators for free.
                vb = [bigp.tile([128, 264], BF, name=f"vb{tc_}", tag=f"vb{tc_}") for tc_ in range(2)]
                vb64 = [bigp.tile([128, 264], BF, name=f"vb64_{tc_}", tag=f"vb64_{tc_}") for tc_ in range(8)]
                for tc_ in range(2):
                    ps = psM.tile([128, D], F32, name="mm", tag="mm")
                    for kc in range(2):
                        nc.tensor.matmul(
                            ps[:], lhsT=xTall[:, kc * L + tc_ * 128:kc * L + (tc_ + 1) * 128],
                            rhs=s_wv[:, kc * D:(kc + 1) * D],
                            start=(kc == 0), stop=(kc == 1))
                    nc.vector.tensor_copy(
                        _bcast(vb[tc_][0:128, 0:1], 0, [[33, 8], [1, 32]]),
                        ps[:])
                    nc.gpsimd.memset(
                        _bcast(vb[tc_][0:128, 0:1], 32, [[33, 8]]), 1.0)
                for tc_ in range(8):
                    rows = 128 if tc_ < 7 else 64
                    ps = psM.tile([128, D], F32, name="mm", tag="mm")
                    for kc in range(2):
                        nc.tensor.matmul(
                            ps[:rows, :],
                            lhsT=xTall[:, kc * L + 64 + tc_ * 128:kc * L + 64 + tc_ * 128 + rows],
                            rhs=s_wv[:, kc * D:(kc + 1) * D],
                            start=(kc == 0), stop=(kc == 1))
                    nc.vector.tensor_copy(
                        _bcast(vb64[tc_][0:rows, 0:1], 0, [[33, 8], [1, 32]]),
                        ps[:rows, :])
                    nc.gpsimd.memset(
                        _bcast(vb64[tc_][0:rows, 0:1], 32, [[33, 8]]), 1.0)

                # ---- attention --------------------------------------------
                # Oall col = dc*1024 + tc*128 + (d - dc*128) so ONE batched
                # transpose produces OTall (see xb_all comment).
                Oall = bigp.tile([128, 8 * D], BF, name="Oall", tag="Oall")
                for b in range(NB):
                    for qc in range(2):
                        lo, hi = _win(b, qc)
                        w = hi - lo
                        wpad = -(-w // 128) * 128
                        nch = wpad // 128
                        qlo = b * 256 + qc * 128
                        # P head-major, 512-stride: col h*512 + j (key j of
                        # head h; j < 320, [320,512) garbage). Contiguous in
                        # j, so mask-mult is one DVE op per 4-head wave, and
                        # exp->mult->transpose->AV pipelines in 2 waves.
                        Pall = bigp.tile([128, 8 * 512], BF, name="Pall", tag="Pall")
                        PT = bigp.tile([128, 8 * 512], BF, name="PT", tag="PT")
                        Ou = psO.tile([128, 264], F32, name="Ou", tag="Ou")
                        if b > 0:
                            Praw = bigp.tile([128, 8 * 320], BF, name="Praw", tag="Praw")
                        for h0 in (0, 4):
                            for h in range(h0, h0 + 4):
                                hc, hr = h // 4, (h % 4) * 32
                                if h % 4 == 3:
                                    qsl = q3[hc][0:32, qlo:qlo + 128]
                                    ksl = k3[hc][0:32, lo:hi]
                                else:
                                    qsl = qT[hc][hr:hr + 32, qlo:qlo + 128]
                                    ksl = kT[hc][hr:hr + 32, lo:hi]
                                S = psS.tile([128, 320], F32, name="S", tag="S")
                                if b == 0:
                                    # causal mask via identity-matmul inject,
                                    # then QK accumulates; exp -> Pall direct.
                                    mask_ap = (s_mb0 if qc == 0 else s_mb1)[:]
                                    nc.tensor.matmul(
                                        S[:, :wpad], lhsT=s_i128[:], rhs=mask_ap,
                                        start=True, stop=False)
                                    nc.tensor.matmul(
                                        S[:, :w], lhsT=qsl, rhs=ksl,
                                        start=False, stop=True)
                                    nc.scalar.activation(
                                        out=Pall[:, h * 512:h * 512 + wpad],
                                        in_=S[:, :wpad], func=AF.Exp)
                                else:
                                    # no mask inject: raw QK, exp, then mult
                                    # by 0/1 mask (masked -> exp(S)*0 = 0)
                                    nc.tensor.matmul(
                                        S[:], lhsT=qsl, rhs=ksl,
                                        start=True, stop=True)
                                    nc.scalar.activation(
                                        out=Praw[:, h * 320:(h + 1) * 320],
                                        in_=S[:], func=AF.Exp)
                            if b > 0:
                                nc.vector.tensor_tensor(
                                    out=_bcast(Pall[0:128, 0:1], h0 * 512,
                                               [[512, 4], [1, 320]]),
                                    in0=_bcast(Praw[0:128, 0:1], h0 * 320,
                                               [[320, 4], [1, 320]]),
                                    in1=_bcast(s_m01[qc][0:128, 0:1], 0,
                                               [[0, 4], [1, 320]]),
                                    op=OP.mult)
                            nc.sync.dma_start_transpose(
                                out=_bcast(PT[0:128, 0:1], h0 * 512,
                                           [[128, 16], [1, 128]]),
                                in_=Pall[:, h0 * 512:(h0 + 4) * 512])
                            for h in range(h0, h0 + 4):
                                for ci in range(nch):
                                    vr = min(128, w - ci * 128)
                                    if b == 0:
                                        vt = vb[ci]
                                    else:
                                        vt = vb64[(lo + ci * 128 - 64) // 128]
                                    nc.tensor.matmul(
                                        Ou[:, h * 33:(h + 1) * 33],
                                        lhsT=PT[0:vr, (h * 4 + ci) * 128:(h * 4 + ci) * 128 + 128],
                                        rhs=vt[0:vr, h * 33:(h + 1) * 33],
                                        start=(ci == 0), stop=(ci == nch - 1))
                        rec = smp.tile([128, 8], F32, name="rec", tag="rec")
                        nc.vector.reciprocal(
                            rec[:], _bcast(Ou[0:128, 0:1], 32, [[33, 8]]))
                        tc_o = 2 * b + qc
                        nc.vector.tensor_tensor(
                            out=_bcast(Oall[0:128, 0:1], tc_o * 128,
                                       [[1024, 2], [32, 4], [1, 32]]),
                            in0=_bcast(Ou[0:128, 0:1], 0,
                                       [[132, 2], [33, 4], [1, 32]]),
                            in1=_bcast(rec[0:128, 0:1], 0,
                                       [[4, 2], [1, 4], [0, 32]]),
                            op=OP.mult)

                OTall = bigp.tile([128, 2 * L], BF, name="OTall", tag="OTall")
                nc.scalar.dma_start_transpose(
                    out=_bcast(OTall[0:128, 0:1], 0, [[128, 16], [1, 128]]),
                    in_=Oall[:])

                # ---- Wo + residual + LN1 ----------------------------------
                z1n_all = bigp.tile([128, 8 * D], BF, name="z1n_all", tag="z1n_all")
                for tc_ in range(8):
                    ps = psM.tile([128, D], F32, name="mm", tag="mm")
                    for dc in range(2):
                        nc.tensor.matmul(
                            ps[:], lhsT=OTall[:, dc * L + tc_ * 128:dc * L + (tc_ + 1) * 128],
                            rhs=s_wo[:, dc * D:(dc + 1) * D],
                            start=(dc == 0), stop=(dc == 1))
                    z1 = smp.tile([128, D], F32, name="z1", tag="z1")
                    nc.vector.tensor_tensor(out=z1[:], in0=ps[:], in1=xt[tc_][:],
                                            op=OP.add)
                    st6 = smp.tile([128, 6], F32, name="st6", tag="st6")
                    st2 = smp.tile([128, 2], F32, name="st2", tag="st2")
                    nc.vector.bn_stats(st6[:], z1[:])
                    nc.vector.bn_aggr(st2[:], st6[:])
                    std = smp.tile([128, 1], F32, name="std", tag="std")
                    nc.scalar.activation(out=std[:], in_=st2[:, 1:2], func=AF.Sqrt,
                                         bias=s_eps[:, 0:1])
                    rstd = smp.tile([128, 1], F32, name="rstd", tag="rstd")
                    nc.vector.reciprocal(rstd[:], std[:])
                    nc.vector.tensor_scalar(
                        out=_bcast(z1n_all[0:128, 0:1], tc_ * 128,
                                   [[1024, 2], [1, 128]]),
                        in0=z1[:],
                        scalar1=st2[:, 0:1],
                        scalar2=rstd[:], op0=OP.subtract, op1=OP.mult)

                z1nTall = bigp.tile([128, 2 * L], BF, name="z1nTall", tag="z1nTall")
                nc.scalar.dma_start_transpose(
                    out=_bcast(z1nTall[0:128, 0:1], 0, [[128, 16], [1, 128]]),
                    in_=z1n_all[:])

                # ---- MLP up (h.T orientation) + fused bias+relu -----------
                # h.T = W1eff.T-chunks @ z1nT; relu(x + b1) with b1 per-
                # partition in this orientation -> no r transpose needed.
                rTall = bigp.tile([128, 4 * L], BF, name="rTall", tag="rTall")
                for fc in range(4):
                    for hf in range(2):
                        ps = psM.tile([128, 512], F32, name="mm", tag="mm")
                        for dc in range(2):
                            nc.tensor.matmul(
                                ps[:],
                                lhsT=s_w1[:, dc * F + fc * 128:dc * F + (fc + 1) * 128],
                                rhs=z1nTall[:, dc * L + hf * 512:dc * L + (hf + 1) * 512],
                                start=(dc == 0), stop=(dc == 1))
                        dst = rTall[:, fc * L + hf * 512:fc * L + (hf + 1) * 512]
                        if (fc + hf) % 2 == 0:
                            nc.scalar.activation(out=dst, in_=ps[:], func=AF.Relu,
                                                 bias=s_b1col[:, fc:fc + 1])
                        else:
                            nc.vector.tensor_scalar(
                                out=dst, in0=ps[:], scalar1=s_b1col[:, fc:fc + 1],
                                scalar2=0.0, op0=OP.add, op1=OP.max)

                # ---- MLP down + injects + LN2 + out -----------------------
                ot_all = bigp.tile([128, 8 * D], F32, name="ot_all", tag="ot_all")
                for tc_ in range(8):
                    ps = psM.tile([128, D], F32, name="mm", tag="mm")
                    for fc in range(4):
                        nc.tensor.matmul(
                            ps[:], lhsT=rTall[:, fc * L + tc_ * 128:fc * L + (tc_ + 1) * 128],
                            rhs=s_w2[:, fc * D:(fc + 1) * D],
                            start=(fc == 0), stop=False)
                    for dc in range(2):
                        nc.tensor.matmul(
                            ps[:], lhsT=z1nTall[:, dc * L + tc_ * 128:dc * L + (tc_ + 1) * 128],
                            rhs=s_dg1[dc][:], start=False, stop=False)
                    nc.tensor.matmul(ps[:], lhsT=s_ones1[:], rhs=s_browz[:],
                                     start=False, stop=True)
                    st6 = smp.tile([128, 6], F32, name="st6", tag="st6")
                    st2 = smp.tile([128, 2], F32, name="st2", tag="st2")
                    nc.vector.bn_stats(st6[:], ps[:])
                    nc.vector.bn_aggr(st2[:], st6[:])
                    std = smp.tile([128, 1], F32, name="std", tag="std")
                    nc.scalar.activation(out=std[:], in_=st2[:, 1:2], func=AF.Sqrt,
                                         bias=s_eps[:, 0:1])
                    rstd = smp.tile([128, 1], F32, name="rstd", tag="rstd")
                    nc.vector.reciprocal(rstd[:], std[:])
                    t1 = smp.tile([128, D], F32, name="t1", tag="t1")
                    nc.vector.scalar_tensor_tensor(
                        out=t1[:], in0=ps[:], scalar=st2[:, 0:1], in1=s_g2b[:],
                        op0=OP.subtract, op1=OP.mult)
                    nc.vector.scalar_tensor_tensor(
                        out=ot_all[:, tc_ * D:(tc_ + 1) * D], in0=t1[:],
                        scalar=rstd[:], in1=s_b2gb[:],
                        op0=OP.mult, op1=OP.add)
                nc.sync.dma_start(
                    out=_mkap(out[s, 0:1, 0:1], 0,
                              [[D, 128], [128 * D, 8], [1, D]]),
                    in_=ot_all[:])

    _split_sync_waits(nc)
    _CACHE["nc"] = nc
    return nc


def _in_maps(X, Wq, Wk, Wv, Wo, ln1_g, ln1_b, W1, b1, W2, b2, ln2_g, ln2_b):
    X = np.asarray(X, dtype=np.float32)
    f32 = lambda a: np.asarray(a, dtype=np.float32)
    Wq, Wk, Wv, Wo = f32(Wq), f32(Wk), f32(Wv), f32(Wo)
    W1, W2 = f32(W1), f32(W2)
    ln1_g, ln1_b, b1, b2 = f32(ln1_g), f32(ln1_b), f32(b1), f32(b2)
    ln2_g, ln2_b = f32(ln2_g), f32(ln2_b)

    bf = ml_dtypes.bfloat16
    w1eff = (ln1_g[:, None] * W1)
    b1eff = (b1 + ln1_b @ W1)
    dg1 = np.zeros((2, 128, D), np.float32)
    for c in range(2):
        for i in range(128):
            dg1[c, i, c * 128 + i] = ln1_g[c * 128 + i]
    m2d = _mask2d()
    mt_core = np.stack([m2d[0:128, 64:384], m2d[128:256, 192:512]])
    m01v = (mt_core == 0.0).astype(np.float32)  # [2, 128, 320] 0/1
    cm = _causal_mask()
    statics = {
        "wq": (Wq * (DH ** -0.5)).astype(bf),
        "wk": Wk.astype(bf),
        "wv": Wv.astype(bf),
        "wo": Wo.astype(bf),
        "w1": w1eff.astype(bf),
        "w2": W2.astype(bf),
        "dg1": dg1.astype(bf),
        "b1col": b1eff.reshape(4, 128).T.astype(np.float32).copy(),
        "browz": (ln1_b + b2)[None, :].astype(bf),
        "g2b": np.tile(ln2_g[None, :], (128, 1)).astype(np.float32),
        "b2gb": np.tile(ln2_b[None, :], (128, 1)).astype(np.float32),
        "i128": np.eye(128).astype(bf),
        "ones1": np.ones((1, 128)).astype(bf),
        "m01": m01v.astype(bf),
        "mb0": cm[0:128, 0:128].astype(bf),
        "mb1": cm[128:256, 0:256].astype(bf),
    }

    in_maps = []
    for i in range(N_CORES):
        m = {"X": X[i * SPC:(i + 1) * SPC]}
        m.update(statics)
        in_maps.append(m)
    return in_maps


def kernel(**inputs):
    from concourse.bass_utils import run_bass_kernel_spmd
    nc = _build()
    res = run_bass_kernel_spmd(nc, _in_maps(**inputs), list(range(N_CORES)))
    return np.concatenate([res.results[i]["OUT"] for i in range(N_CORES)], axis=0)


def kernel_profiled(tmpdir=None, **inputs):
    from concourse.bass_utils import run_bass_kernel_spmd
    nc = _build()
    res = run_bass_kernel_spmd(nc, _in_maps(**inputs), list(range(N_CORES)),
                               trace=True, tmpdir=tmpdir)
    out = np.concatenate([res.results[i]["OUT"] for i in range(N_CORES)], axis=0)
    return out, res



# revision 17
# speedup vs baseline: 1.1251x; 1.1251x over previous
"""Trainium2 Bass kernel for nn_DecoderLayer_11424613007924.

DecoderLayer: block-sparse attention (BLEN=256, causal first block,
2D-local windowed tail blocks) + LayerNorm + MLP, fp32 I/O.

Sharding: data-parallel over batch. 32 samples -> 8 NeuronCores x 4.

Per-core dataflow (per sample):
  X [1024,256] --(cast bf16, DMA-transpose)--> X.T
  Q.T/K.T = W.T @ X.T (PE, bf16);  V = X.T-stationary @ Wv (token-major)
  per (block b, query-chunk qc, head h):
      S  = q.T-chunk.T @ k.T-window (+ mask via identity-matmul inject, PSUM)
      P  = exp(S)  (ACT, accum_out -> softmax denominators)
      P.T via DMA-transpose; O_u = P.T.T @ v (PE); O = O_u * recip(den) (DVE)
  y = O.T-stationary @ Wo; z1 = y + X (fp32)
  LN1 via bn_stats; z1n = (z1-m)*rstd (g1 folded into W1 host-side)
  h = z1n.T @ W1eff + b1eff (ones-row inject); r = relu(h)
  z2 = r.T @ W2 + (ln1_b+b2) inject + z1n*g1 inject (diag(g1) matmul)
  out = (z2-m2)*g2b*rstd2 + b2gb  (scalar_tensor_tensor x2)

Numerics: matmul operands bf16 (PSUM fp32 accum); residual stream,
LN stats, softmax denominators fp32. Masks use -30 instead of -1e9
(exp(-30)*512 ~ 5e-11 relative contamination).
"""
import numpy as np
import ml_dtypes

import concourse.bass as bass
import concourse.mybir as mybir
import concourse.tile as tile
from bass_rust import ScopedClock

BF = mybir.dt.bfloat16
F32 = mybir.dt.float32
AF = mybir.ActivationFunctionType
OP = mybir.AluOpType

N_CORES = 8
B, L, D = 32, 1024, 256
H, DH, F = 8, 32, 512
BLEN = 256
SPC = B // N_CORES  # samples per core
NB = L // BLEN      # 4 blocks
NEG = -30.0
EPS = 1e-6

# ---------------------------------------------------------------- fixups ---
# This container's walrus build rejects instructions carrying >1 semaphore
# wait. Split extra waits onto same-engine NOPs after Tile scheduling.


def _split_sync_waits(nc):
    for fn in nc.m.functions:
        for bb in fn.blocks:
            insts = bb.instructions
            if not insts:
                continue
            new_list = []
            n_split = 0
            for inst in insts:
                si = inst.sync_info
                waits = list(si.on_wait) if (si and si.on_wait) else []
                if len(waits) > 1:
                    si.on_wait = waits[:1]
                    for w in waits[1:]:
                        nop = nc.engines[inst.engine].nop()
                        for f2 in nc.m.functions:
                            for b2 in f2.blocks:
                                l2 = b2.instructions
                                if l2 and l2[-1] is nop.ins:
                                    l2.pop()
                        nop.ins.sync_info = mybir.SyncInfo(on_wait=[w], on_update=[])
                        new_list.append(nop.ins)
                        n_split += 1
                new_list.append(inst)
            if n_split:
                bb.instructions = new_list


def _patched_drain_and_barrier(self, tick_clock, wait_clock):
    nc = self.nc
    probe = nc.sync.nop()
    wait_clock.add_sem_waits(probe.ins, ScopedClock({None: tick_clock.global_clock}))
    nc.sync.drain()
    nc.all_engine_barrier()
    assert self.sems is not None
    popped = nc._tile_sem_poison_stack.pop()
    assert popped is self._sem_poison
    nc.clear_and_free_semaphores(list(self.sems.allocated().values()))
    nc.all_engine_barrier()


tile.TileContext._drain_and_barrier = _patched_drain_and_barrier


# ------------------------------------------------------------- host prep ---

def _mask2d(blen=BLEN, h=32, win=6):
    cp = np.arange(blen, 2 * blen)[:, None]
    op = np.arange(2 * blen)[None, :]
    causal = op <= cp
    ch, cw = cp // h, cp % h
    oh, ow = op // h, op % h
    ok = causal & (np.abs(ch - oh) <= win) & (np.abs(cw - ow) <= win)
    return np.where(ok, 0.0, NEG).astype(np.float32)


def _causal_mask(blen=BLEN):
    return np.where(np.tril(np.ones((blen, blen), bool)), 0.0, NEG).astype(np.float32)


def _bcast(ap, offset_extra, plist):
    """AP with explicit [step,count] free dims appended to partition dim."""
    return bass.AP(tensor=ap.tensor, offset=ap.offset + offset_extra,
                   ap=[list(ap.ap[0])] + plist)


def _mkap(ap, offset_extra, dims):
    """AP with fully explicit [step,count] dims (incl. partition dim)."""
    return bass.AP(tensor=ap.tensor, offset=ap.offset + offset_extra, ap=dims)


def _win(b, qc):
    """Key window (absolute token range) for block b, query chunk qc."""
    if b == 0:
        return 0, 128 * (qc + 1)
    a = (b - 1) * 256 + 64 + 128 * qc
    return a, a + 320




_CACHE = {}

# tuning knobs (read at build time)
KNOBS = {"psS": 3, "psO": 2, "psM": 3, "big": 2, "small": 2}


def _build():
    if "nc" in _CACHE:
        return _CACHE["nc"]
    nc = bass.Bass(target_bir_lowering=False)

    xin = nc.declare_dram_parameter("X", [SPC, L, D], F32, isOutput=False)
    out = nc.declare_dram_parameter("OUT", [SPC, L, D], F32, isOutput=True)
    wq = nc.declare_dram_parameter("wq", [D, D], BF, isOutput=False)
    wk = nc.declare_dram_parameter("wk", [D, D], BF, isOutput=False)
    wv = nc.declare_dram_parameter("wv", [D, D], BF, isOutput=False)
    wo = nc.declare_dram_parameter("wo", [D, D], BF, isOutput=False)
    w1 = nc.declare_dram_parameter("w1", [D, F], BF, isOutput=False)
    w2 = nc.declare_dram_parameter("w2", [F, D], BF, isOutput=False)
    dg1 = nc.declare_dram_parameter("dg1", [2, 128, D], BF, isOutput=False)
    b1col = nc.declare_dram_parameter("b1col", [128, 4], F32, isOutput=False)
    browz = nc.declare_dram_parameter("browz", [1, D], BF, isOutput=False)
    g2b = nc.declare_dram_parameter("g2b", [128, D], F32, isOutput=False)
    b2gb = nc.declare_dram_parameter("b2gb", [128, D], F32, isOutput=False)
    i128 = nc.declare_dram_parameter("i128", [128, 128], BF, isOutput=False)
    ones1 = nc.declare_dram_parameter("ones1", [1, 128], BF, isOutput=False)
    m01 = nc.declare_dram_parameter("m01", [2, 128, 320], BF, isOutput=False)
    mb0 = nc.declare_dram_parameter("mb0", [128, 128], BF, isOutput=False)
    mb1 = nc.declare_dram_parameter("mb1", [128, 256], BF, isOutput=False)

    with tile.TileContext(nc) as tc:
        with (
            tc.tile_pool(name="static", bufs=1) as st,
            tc.tile_pool(name="big", bufs=KNOBS["big"]) as bigp,
            tc.tile_pool(name="small", bufs=KNOBS["small"]) as smp,
            tc.tile_pool(name="psS", bufs=KNOBS["psS"], space="PSUM") as psS,
            tc.tile_pool(name="psO", bufs=KNOBS["psO"], space="PSUM") as psO,
            tc.tile_pool(name="psM", bufs=KNOBS["psM"], space="PSUM") as psM,
        ):
            # ---- statics
            # weights stored K-chunked: chunk kc lives at cols [kc*N, (kc+1)*N)
            s_wq = st.tile([128, 2 * D], BF, name="wq", tag="wq")
            s_wk = st.tile([128, 2 * D], BF, name="wk", tag="wk")
            s_wv = st.tile([128, 2 * D], BF, name="wv", tag="wv")
            s_wo = st.tile([128, 2 * D], BF, name="wo", tag="wo")
            s_w1 = st.tile([128, 2 * F], BF, name="w1", tag="w1")
            s_w2 = st.tile([128, 4 * D], BF, name="w2", tag="w2")
            s_dg1 = [st.tile([128, D], BF, name=f"dg1{c}", tag=f"dg1{c}") for c in range(2)]
            s_b1col = st.tile([128, 4], F32, name="b1col", tag="b1col")
            s_browz = st.tile([1, D], BF, name="browz", tag="browz")
            s_g2b = st.tile([128, D], F32, name="g2b", tag="g2b")
            s_b2gb = st.tile([128, D], F32, name="b2gb", tag="b2gb")
            s_i128 = st.tile([128, 128], BF, name="i128", tag="i128")
            s_ones1 = st.tile([1, 128], BF, name="ones1", tag="ones1")
            s_m01 = [st.tile([128, 320], BF, name=f"m01_{qc}", tag=f"m01_{qc}") for qc in range(2)]
            s_mb0 = st.tile([128, 128], BF, name="mb0", tag="mb0")
            s_eps = st.tile([128, 1], F32, name="eps", tag="eps")
            s_mb1 = st.tile([128, 256], BF, name="mb1", tag="mb1")
            # qk-projection weights first: sample 0's first matmuls need them
            for dst, dsrc, nch_, w_ in [
                (s_wq, wq, 2, D), (s_wk, wk, 2, D), (s_wv, wv, 2, D),
                (s_wo, wo, 2, D), (s_w1, w1, 2, F), (s_w2, w2, 4, D),
            ]:
                for kc in range(nch_):
                    nc.sync.dma_start(out=dst[:, kc * w_:(kc + 1) * w_],
                                      in_=dsrc[kc * 128:(kc + 1) * 128, :])
            for dst, dsrc in [
                (s_mb0, mb0), (s_mb1, mb1), (s_i128, i128), (s_ones1, ones1),
                (s_b1col, b1col), (s_browz, browz),
                (s_g2b, g2b), (s_b2gb, b2gb),
            ]:
                nc.sync.dma_start(out=dst[:], in_=dsrc[:])
            nc.vector.memset(s_eps[:], EPS)
            for c in range(2):
                nc.sync.dma_start(out=s_dg1[c][:], in_=dg1[c])
                nc.sync.dma_start(out=s_m01[c][:], in_=m01[c])

            # weight chunk kc (K-rows kc*128..) cols [mlo,mhi) of a w_-wide chunk
            def wch(t, kc, w_, mlo, mhi):
                return t[:, kc * w_ + mlo:kc * w_ + mhi]

            # ---- X load / cast / transpose, software-pipelined -------------
            # Emitted one sample ahead (prefetch) on the scalar+vector queues
            # so the chain never sits behind the current sample's sync-queue
            # transposes (in-order queues = head-of-line blocking).
            xt_c, xT_c = {}, {}

            def prefetch(s):
                xt_all = bigp.tile([128, 8 * D], F32, name="xt_all", tag="xt_all")
                nc.scalar.dma_start(
                    out=xt_all[:],
                    in_=_mkap(xin[s, 0:1, 0:1], 0,
                              [[D, 128], [128 * D, 8], [1, D]]))
                # xb_all col = dc*1024 + tc*128 + p' (C-major, C = dc*8+tc)
                # so ONE batched transpose yields xTall col = C*128 + j
                #   = dc*1024 + tc*128 + j  (the layout consumers expect)
                xb_all = bigp.tile([128, 8 * D], BF, name="xb_all", tag="xb_all")
                nc.vector.tensor_copy(
                    _bcast(xb_all[0:128, 0:1], 0,
                           [[128, 8], [1024, 2], [1, 128]]),
                    _bcast(xt_all[0:128, 0:1], 0,
                           [[256, 8], [128, 2], [1, 128]]))
                xTall = bigp.tile([128, 2 * L], BF, name="xTall", tag="xTall")
                nc.scalar.dma_start_transpose(
                    out=_bcast(xTall[0:128, 0:1], 0, [[128, 16], [1, 128]]),
                    in_=xb_all[:])
                xt_c[s] = [xt_all[:, tc_ * D:(tc_ + 1) * D] for tc_ in range(8)]
                xT_c[s] = xTall

            prefetch(0)
            for s in range(SPC):
                if s + 1 < SPC:
                    prefetch(s + 1)
                xt, xTall = xt_c.pop(s), xT_c.pop(s)

                # ---- Q.T / K.T (d-major) and V (token-major) ---------------
                qT = [bigp.tile([128, L], BF, name=f"qT{mc}", tag=f"qT{mc}") for mc in range(2)]
                kT = [bigp.tile([128, L], BF, name=f"kT{mc}", tag=f"kT{mc}") for mc in range(2)]
                for wt, dstl in ((s_wq, qT), (s_wk, kT)):
                    for mc in range(2):
                        for hf in range(2):
                            ps = psM.tile([128, 512], F32, name="mm", tag="mm")
                            for kc in range(2):
                                nc.tensor.matmul(
                                    ps[:], lhsT=wch(wt, kc, D, mc * 128, mc * 128 + 128),
                                    rhs=xTall[:, kc * L + hf * 512:kc * L + (hf + 1) * 512],
                                    start=(kc == 0), stop=(kc == 1))
                            nc.vector.tensor_copy(
                                dstl[mc][:, hf * 512:(hf + 1) * 512], ps[:])
                # head h%4==3 sits at base partition 96 (invalid for matmul
                # operands); extract to offset-0 tiles via SBUF->SBUF DMA
                q3 = [smp.tile([32, L], BF, name=f"q3_{hc}", tag=f"q3_{hc}") for hc in range(2)]
                k3 = [smp.tile([32, L], BF, name=f"k3_{hc}", tag=f"k3_{hc}") for hc in range(2)]
                for hc in range(2):
                    nc.sync.dma_start(out=q3[hc][:], in_=qT[hc][96:128, :])
                    nc.sync.dma_start(out=k3[hc][:], in_=kT[hc][96:128, :])
                # vb: 0-aligned V token-chunks (block0 needs tokens [0,256)).
                # vb64: 64-shifted chunks, vb64[t] rows <-> tokens 64+t*128..,
                # aligning tail-window AV chunks to a single matmul each.
                # Layout [128, 8*33]: head h at cols h*33..h*33+32, col h*33+32
                # is ones -> AV matmul emits softmax denominators for free.
                vb = [bigp.tile([128, 264], BF, name=f"vb{tc_}", tag=f"vb{tc_}") for tc_ in range(2)]
                vb64 = [bigp.tile([128, 264], BF, name=f"vb64_{tc_}", tag=f"vb64_{tc_}") for tc_ in range(8)]
                for tc_ in range(2):
                    ps = psM.tile([128, D], F32, name="mm", tag="mm")
                    for kc in range(2):
                        nc.tensor.matmul(
                            ps[:], lhsT=xTall[:, kc * L + tc_ * 128:kc * L + (tc_ + 1) * 128],
                            rhs=s_wv[:, kc * D:(kc + 1) * D],
                            start=(kc == 0), stop=(kc == 1))
                    nc.vector.tensor_copy(
                        _bcast(vb[tc_][0:128, 0:1], 0, [[33, 8], [1, 32]]),
                        ps[:])
                    nc.gpsimd.memset(
                        _bcast(vb[tc_][0:128, 0:1], 32, [[33, 8]]), 1.0)
                for tc_ in range(8):
                    rows = 128 if tc_ < 7 else 64
                    ps = psM.tile([128, D], F32, name="mm", tag="mm")
                    for kc in range(2):
                        nc.tensor.matmul(
                            ps[:rows, :],
                            lhsT=xTall[:, kc * L + 64 + tc_ * 128:kc * L + 64 + tc_ * 128 + rows],
                            rhs=s_wv[:, kc * D:(kc + 1) * D],
                            start=(kc == 0), stop=(kc == 1))
                    nc.vector.tensor_copy(
                        _bcast(vb64[tc_][0:rows, 0:1], 0, [[33, 8], [1, 32]]),
                        ps[:rows, :])
                    nc.gpsimd.memset(
                        _bcast(vb64[tc_][0:rows, 0:1], 32, [[33, 8]]), 1.0)

                # ---- attention --------------------------------------------
                # Oall col = dc*1024 + tc*128 + (d - dc*128) so ONE batched
                # transpose produces OTall (see xb_all comment).
                Oall = bigp.tile([128, 8 * D], BF, name="Oall", tag="Oall")
                for b in range(NB):
                    for qc in range(2):
                        lo, hi = _win(b, qc)
                        w = hi - lo
                        wpad = -(-w // 128) * 128
                        nch = wpad // 128
                        qlo = b * 256 + qc * 128
                        # P head-major, 384-stride: col h*384 + j (key j of
                        # head h; j < 320 valid, [320,384) garbage whose
                        # transposed rows are excluded by vr=64). Contiguous
                        # in j -> ONE DVE mask-mult; one PT transpose per bq.
                        Pall = bigp.tile([128, 8 * 384], BF, name="Pall", tag="Pall")
                        PT = bigp.tile([128, 8 * 384], BF, name="PT", tag="PT")
                        Ou = psO.tile([128, 264], F32, name="Ou", tag="Ou")
                        if b > 0:
                            Praw = bigp.tile([128, 8 * 320], BF, name="Praw", tag="Praw")
                        for h in range(H):
                            hc, hr = h // 4, (h % 4) * 32
                            if h % 4 == 3:
                                qsl = q3[hc][0:32, qlo:qlo + 128]
                                ksl = k3[hc][0:32, lo:hi]
                            else:
                                qsl = qT[hc][hr:hr + 32, qlo:qlo + 128]
                                ksl = kT[hc][hr:hr + 32, lo:hi]
                            S = psS.tile([128, 320], F32, name="S", tag="S")
                            if b == 0:
                                # causal mask via identity-matmul inject,
                                # then QK accumulates; exp -> Pall direct.
                                mask_ap = (s_mb0 if qc == 0 else s_mb1)[:]
                                nc.tensor.matmul(
                                    S[:, :wpad], lhsT=s_i128[:], rhs=mask_ap,
                                    start=True, stop=False)
                                nc.tensor.matmul(
                                    S[:, :w], lhsT=qsl, rhs=ksl,
                                    start=False, stop=True)
                                # b0: chunk-major (col ci*1024 + h*128 + j) so
                                # the transpose only moves nch*1024 cols
                                nc.scalar.activation(
                                    out=_bcast(Pall[0:128, 0:1], h * 128,
                                               [[1024, nch], [1, 128]]),
                                    in_=S[:, :wpad], func=AF.Exp)
                            else:
                                # no mask inject: raw QK, exp, then mult
                                # by 0/1 mask (masked -> exp(S)*0 = 0)
                                nc.tensor.matmul(
                                    S[:], lhsT=qsl, rhs=ksl,
                                    start=True, stop=True)
                                nc.scalar.activation(
                                    out=Praw[:, h * 320:(h + 1) * 320],
                                    in_=S[:], func=AF.Exp)
                        if b > 0:
                            nc.vector.tensor_tensor(
                                out=_bcast(Pall[0:128, 0:1], 0,
                                           [[384, 8], [1, 320]]),
                                in0=_bcast(Praw[0:128, 0:1], 0,
                                           [[320, 8], [1, 320]]),
                                in1=_bcast(s_m01[qc][0:128, 0:1], 0,
                                           [[0, 8], [1, 320]]),
                                op=OP.mult)
                        if b == 0:
                            nc.sync.dma_start_transpose(
                                out=_bcast(PT[0:128, 0:1], 0,
                                           [[128, 8 * nch], [1, 128]]),
                                in_=Pall[:, 0:nch * 1024])
                        else:
                            nc.sync.dma_start_transpose(
                                out=_bcast(PT[0:128, 0:1], 0,
                                           [[128, 24], [1, 128]]),
                                in_=Pall[:])
                        for h in range(H):
                            for ci in range(nch):
                                vr = min(128, w - ci * 128)
                                if b == 0:
                                    vt = vb[ci]
                                    blk = ci * 8 + h
                                else:
                                    vt = vb64[(lo + ci * 128 - 64) // 128]
                                    blk = h * 3 + ci
                                nc.tensor.matmul(
                                    Ou[:, h * 33:(h + 1) * 33],
                                    lhsT=PT[0:vr, blk * 128:(blk + 1) * 128],
                                    rhs=vt[0:vr, h * 33:(h + 1) * 33],
                                    start=(ci == 0), stop=(ci == nch - 1))
                        rec = smp.tile([128, 8], F32, name="rec", tag="rec")
                        nc.vector.reciprocal(
                            rec[:], _bcast(Ou[0:128, 0:1], 32, [[33, 8]]))
                        tc_o = 2 * b + qc
                        nc.vector.tensor_tensor(
                            out=_bcast(Oall[0:128, 0:1], tc_o * 128,
                                       [[1024, 2], [32, 4], [1, 32]]),
                            in0=_bcast(Ou[0:128, 0:1], 0,
                                       [[132, 2], [33, 4], [1, 32]]),
                            in1=_bcast(rec[0:128, 0:1], 0,
                                       [[4, 2], [1, 4], [0, 32]]),
                            op=OP.mult)

                OTall = bigp.tile([128, 2 * L], BF, name="OTall", tag="OTall")
                nc.scalar.dma_start_transpose(
                    out=_bcast(OTall[0:128, 0:1], 0, [[128, 16], [1, 128]]),
                    in_=Oall[:])

                # ---- Wo + residual + LN1 ----------------------------------
                z1n_all = bigp.tile([128, 8 * D], BF, name="z1n_all", tag="z1n_all")
                for tc_ in range(8):
                    ps = psM.tile([128, D], F32, name="mm", tag="mm")
                    for dc in range(2):
                        nc.tensor.matmul(
                            ps[:], lhsT=OTall[:, dc * L + tc_ * 128:dc * L + (tc_ + 1) * 128],
                            rhs=s_wo[:, dc * D:(dc + 1) * D],
                            start=(dc == 0), stop=(dc == 1))
                    z1 = smp.tile([128, D], F32, name="z1", tag="z1")
                    nc.vector.tensor_tensor(out=z1[:], in0=ps[:], in1=xt[tc_][:],
                                            op=OP.add)
                    st6 = smp.tile([128, 6], F32, name="st6", tag="st6")
                    st2 = smp.tile([128, 2], F32, name="st2", tag="st2")
                    nc.vector.bn_stats(st6[:], z1[:])
                    nc.vector.bn_aggr(st2[:], st6[:])
                    std = smp.tile([128, 1], F32, name="std", tag="std")
                    nc.scalar.activation(out=std[:], in_=st2[:, 1:2], func=AF.Sqrt,
                                         bias=s_eps[:, 0:1])
                    rstd = smp.tile([128, 1], F32, name="rstd", tag="rstd")
                    nc.vector.reciprocal(rstd[:], std[:])
                    nc.vector.tensor_scalar(
                        out=_bcast(z1n_all[0:128, 0:1], tc_ * 128,
                                   [[1024, 2], [1, 128]]),
                        in0=z1[:],
                        scalar1=st2[:, 0:1],
                        scalar2=rstd[:], op0=OP.subtract, op1=OP.mult)

                z1nTall = bigp.tile([128, 2 * L], BF, name="z1nTall", tag="z1nTall")
                nc.scalar.dma_start_transpose(
                    out=_bcast(z1nTall[0:128, 0:1], 0, [[128, 16], [1, 128]]),
                    in_=z1n_all[:])

                # ---- MLP up (h.T orientation) + fused bias+relu -----------
                # h.T = W1eff.T-chunks @ z1nT; relu(x + b1) with b1 per-
                # partition in this orientation -> no r transpose needed.
                rTall = bigp.tile([128, 4 * L], BF, name="rTall", tag="rTall")
                for fc in range(4):
                    for hf in range(2):
                        ps = psM.tile([128, 512], F32, name="mm", tag="mm")
                        for dc in range(2):
                            nc.tensor.matmul(
                                ps[:],
                                lhsT=s_w1[:, dc * F + fc * 128:dc * F + (fc + 1) * 128],
                                rhs=z1nTall[:, dc * L + hf * 512:dc * L + (hf + 1) * 512],
                                start=(dc == 0), stop=(dc == 1))
                        dst = rTall[:, fc * L + hf * 512:fc * L + (hf + 1) * 512]
                        if (fc + hf) % 2 == 0:
                            nc.scalar.activation(out=dst, in_=ps[:], func=AF.Relu,
                                                 bias=s_b1col[:, fc:fc + 1])
                        else:
                            nc.vector.tensor_scalar(
                                out=dst, in0=ps[:], scalar1=s_b1col[:, fc:fc + 1],
                                scalar2=0.0, op0=OP.add, op1=OP.max)

                # ---- MLP down + injects + LN2 + out -----------------------
                ot_all = bigp.tile([128, 8 * D], F32, name="ot_all", tag="ot_all")
                for tc_ in range(8):
                    ps = psM.tile([128, D], F32, name="mm", tag="mm")
                    for fc in range(4):
                        nc.tensor.matmul(
                            ps[:], lhsT=rTall[:, fc * L + tc_ * 128:fc * L + (tc_ + 1) * 128],
                            rhs=s_w2[:, fc * D:(fc + 1) * D],
                            start=(fc == 0), stop=False)
                    for dc in range(2):
                        nc.tensor.matmul(
                            ps[:], lhsT=z1nTall[:, dc * L + tc_ * 128:dc * L + (tc_ + 1) * 128],
                            rhs=s_dg1[dc][:], start=False, stop=False)
                    nc.tensor.matmul(ps[:], lhsT=s_ones1[:], rhs=s_browz[:],
                                     start=False, stop=True)
                    st6 = smp.tile([128, 6], F32, name="st6", tag="st6")
                    st2 = smp.tile([128, 2], F32, name="st2", tag="st2")
                    nc.vector.bn_stats(st6[:], ps[:])
                    nc.vector.bn_aggr(st2[:], st6[:])
                    std = smp.tile([128, 1], F32, name="std", tag="std")
                    nc.scalar.activation(out=std[:], in_=st2[:, 1:2], func=AF.Sqrt,
                                         bias=s_eps[:, 0:1])
                    rstd = smp.tile([128, 1], F32, name="rstd", tag="rstd")
                    nc.vector.reciprocal(rstd[:], std[:])
                    t1 = smp.tile([128, D], F32, name="t1", tag="t1")
                    nc.vector.scalar_tensor_tensor(
                        out=t1[:], in0=ps[:], scalar=st2[:, 0:1], in1=s_g2b[:],
                        op0=OP.subtract, op1=OP.mult)
                    nc.vector.scalar_tensor_tensor(
                        out=ot_all[:, tc_ * D:(tc_ + 1) * D], in0=t1[:],
                        scalar=rstd[:], in1=s_b2gb[:],
                        op0=OP.mult, op1=OP.add)
                nc.sync.dma_start(
                    out=_mkap(out[s, 0:1, 0:1], 0,
                              [[D, 128], [128 * D, 8], [1, D]]),
                    in_=ot_all[:])

    _split_sync_waits(nc)
    _CACHE["nc"] = nc
    return nc


def _in_maps(X, Wq, Wk, Wv, Wo, ln1_g, ln1_b, W1, b1, W2, b2, ln2_g, ln2_b):
    X = np.asarray(X, dtype=np.float32)
    f32 = lambda a: np.asarray(a, dtype=np.float32)
    Wq, Wk, Wv, Wo = f32(Wq), f32(Wk), f32(Wv), f32(Wo)
    W1, W2 = f32(W1), f32(W2)
    ln1_g, ln1_b, b1, b2 = f32(ln1_g), f32(ln1_b), f32(b1), f32(b2)
    ln2_g, ln2_b = f32(ln2_g), f32(ln2_b)

    bf = ml_dtypes.bfloat16
    w1eff = (ln1_g[:, None] * W1)
    b1eff = (b1 + ln1_b @ W1)
    dg1 = np.zeros((2, 128, D), np.float32)
    for c in range(2):
        for i in range(128):
            dg1[c, i, c * 128 + i] = ln1_g[c * 128 + i]
    m2d = _mask2d()
    mt_core = np.stack([m2d[0:128, 64:384], m2d[128:256, 192:512]])
    m01v = (mt_core == 0.0).astype(np.float32)  # [2, 128, 320] 0/1
    cm = _causal_mask()
    statics = {
        "wq": (Wq * (DH ** -0.5)).astype(bf),
        "wk": Wk.astype(bf),
        "wv": Wv.astype(bf),
        "wo": Wo.astype(bf),
        "w1": w1eff.astype(bf),
        "w2": W2.astype(bf),
        "dg1": dg1.astype(bf),
        "b1col": b1eff.reshape(4, 128).T.astype(np.float32).copy(),
        "browz": (ln1_b + b2)[None, :].astype(bf),
        "g2b": np.tile(ln2_g[None, :], (128, 1)).astype(np.float32),
        "b2gb": np.tile(ln2_b[None, :], (128, 1)).astype(np.float32),
        "i128": np.eye(128).astype(bf),
        "ones1": np.ones((1, 128)).astype(bf),
        "m01": m01v.astype(bf),
        "mb0": cm[0:128, 0:128].astype(bf),
        "mb1": cm[128:256, 0:256].astype(bf),
    }

    in_maps = []
    for i in range(N_CORES):
        m = {"X": X[i * SPC:(i + 1) * SPC]}
        m.update(statics)
        in_maps.append(m)
    return in_maps


def kernel(**inputs):
    from concourse.bass_utils import run_bass_kernel_spmd
    nc = _build()
    res = run_bass_kernel_spmd(nc, _in_maps(**inputs), list(range(N_CORES)))
    return np.concatenate([res.results[i]["OUT"] for i in range(N_CORES)], axis=0)


def kernel_profiled(tmpdir=None, **inputs):
    from concourse.bass_utils import run_bass_kernel_spmd
    nc = _build()
    res = run_bass_kernel_spmd(nc, _in_maps(**inputs), list(range(N_CORES)),
                               trace=True, tmpdir=tmpdir)
    out = np.concatenate([res.results[i]["OUT"] for i in range(N_CORES)], axis=0)
    return out, res



# revision 23
# speedup vs baseline: 1.1394x; 1.0127x over previous
"""Trainium2 Bass kernel for nn_DecoderLayer_11424613007924.

DecoderLayer: block-sparse attention (BLEN=256, causal first block,
2D-local windowed tail blocks) + LayerNorm + MLP, fp32 I/O.

Sharding: data-parallel over batch. 32 samples -> 8 NeuronCores x 4.

Per-core dataflow (per sample):
  X [1024,256] --(cast bf16, DMA-transpose)--> X.T
  Q.T/K.T = W.T @ X.T (PE, bf16);  V = X.T-stationary @ Wv (token-major)
  per (block b, query-chunk qc, head h):
      S  = q.T-chunk.T @ k.T-window (+ mask via identity-matmul inject, PSUM)
      P  = exp(S)  (ACT, accum_out -> softmax denominators)
      P.T via DMA-transpose; O_u = P.T.T @ v (PE); O = O_u * recip(den) (DVE)
  y = O.T-stationary @ Wo; z1 = y + X (fp32)
  LN1 via bn_stats; z1n = (z1-m)*rstd (g1 folded into W1 host-side)
  h = z1n.T @ W1eff + b1eff (ones-row inject); r = relu(h)
  z2 = r.T @ W2 + (ln1_b+b2) inject + z1n*g1 inject (diag(g1) matmul)
  out = (z2-m2)*g2b*rstd2 + b2gb  (scalar_tensor_tensor x2)

Numerics: matmul operands bf16 (PSUM fp32 accum); residual stream,
LN stats, softmax denominators fp32. Masks use -30 instead of -1e9
(exp(-30)*512 ~ 5e-11 relative contamination).
"""
import numpy as np
import ml_dtypes

import concourse.bass as bass
import concourse.mybir as mybir
import concourse.tile as tile
from bass_rust import ScopedClock

BF = mybir.dt.bfloat16
F32 = mybir.dt.float32
AF = mybir.ActivationFunctionType
OP = mybir.AluOpType

N_CORES = 8
B, L, D = 32, 1024, 256
H, DH, F = 8, 32, 512
BLEN = 256
SPC = B // N_CORES  # samples per core
NB = L // BLEN      # 4 blocks
NEG = -30.0
EPS = 1e-6

# ---------------------------------------------------------------- fixups ---
# This container's walrus build rejects instructions carrying >1 semaphore
# wait. Split extra waits onto same-engine NOPs after Tile scheduling.


def _split_sync_waits(nc):
    for fn in nc.m.functions:
        for bb in fn.blocks:
            insts = bb.instructions
            if not insts:
                continue
            new_list = []
            n_split = 0
            for inst in insts:
                si = inst.sync_info
                waits = list(si.on_wait) if (si and si.on_wait) else []
                if len(waits) > 1:
                    si.on_wait = waits[:1]
                    for w in waits[1:]:
                        nop = nc.engines[inst.engine].nop()
                        for f2 in nc.m.functions:
                            for b2 in f2.blocks:
                                l2 = b2.instructions
                                if l2 and l2[-1] is nop.ins:
                                    l2.pop()
                        nop.ins.sync_info = mybir.SyncInfo(on_wait=[w], on_update=[])
                        new_list.append(nop.ins)
                        n_split += 1
                new_list.append(inst)
            if n_split:
                bb.instructions = new_list


def _patched_drain_and_barrier(self, tick_clock, wait_clock):
    nc = self.nc
    probe = nc.sync.nop()
    wait_clock.add_sem_waits(probe.ins, ScopedClock({None: tick_clock.global_clock}))
    nc.sync.drain()
    nc.all_engine_barrier()
    assert self.sems is not None
    popped = nc._tile_sem_poison_stack.pop()
    assert popped is self._sem_poison
    nc.clear_and_free_semaphores(list(self.sems.allocated().values()))
    nc.all_engine_barrier()


tile.TileContext._drain_and_barrier = _patched_drain_and_barrier


# ------------------------------------------------------------- host prep ---

def _mask2d(blen=BLEN, h=32, win=6):
    cp = np.arange(blen, 2 * blen)[:, None]
    op = np.arange(2 * blen)[None, :]
    causal = op <= cp
    ch, cw = cp // h, cp % h
    oh, ow = op // h, op % h
    ok = causal & (np.abs(ch - oh) <= win) & (np.abs(cw - ow) <= win)
    return np.where(ok, 0.0, NEG).astype(np.float32)


def _causal_mask(blen=BLEN):
    return np.where(np.tril(np.ones((blen, blen), bool)), 0.0, NEG).astype(np.float32)


def _bcast(ap, offset_extra, plist):
    """AP with explicit [step,count] free dims appended to partition dim."""
    return bass.AP(tensor=ap.tensor, offset=ap.offset + offset_extra,
                   ap=[list(ap.ap[0])] + plist)


def _mkap(ap, offset_extra, dims):
    """AP with fully explicit [step,count] dims (incl. partition dim)."""
    return bass.AP(tensor=ap.tensor, offset=ap.offset + offset_extra, ap=dims)


def _win(b, qc):
    """Key window (absolute token range) for block b, query chunk qc."""
    if b == 0:
        return 0, 128 * (qc + 1)
    a = (b - 1) * 256 + 64 + 128 * qc
    return a, a + 320




_CACHE = {}

# tuning knobs (read at build time)
KNOBS = {"psS": 3, "psO": 2, "psM": 3, "big": 2, "small": 2}


def _build():
    if "nc" in _CACHE:
        return _CACHE["nc"]
    nc = bass.Bass(target_bir_lowering=False)

    xin = nc.declare_dram_parameter("X", [SPC, L, D], F32, isOutput=False)
    out = nc.declare_dram_parameter("OUT", [SPC, L, D], F32, isOutput=True)
    wq = nc.declare_dram_parameter("wq", [D, D], BF, isOutput=False)
    wk = nc.declare_dram_parameter("wk", [D, D], BF, isOutput=False)
    wv = nc.declare_dram_parameter("wv", [D, D], BF, isOutput=False)
    wo = nc.declare_dram_parameter("wo", [D, D], BF, isOutput=False)
    w1 = nc.declare_dram_parameter("w1", [D, F], BF, isOutput=False)
    w2 = nc.declare_dram_parameter("w2", [F, D], BF, isOutput=False)
    dg1 = nc.declare_dram_parameter("dg1", [2, 128, D], BF, isOutput=False)
    b1col = nc.declare_dram_parameter("b1col", [128, 4], F32, isOutput=False)
    browz = nc.declare_dram_parameter("browz", [1, D], BF, isOutput=False)
    g2b = nc.declare_dram_parameter("g2b", [128, D], F32, isOutput=False)
    b2gb = nc.declare_dram_parameter("b2gb", [128, D], F32, isOutput=False)
    ones1 = nc.declare_dram_parameter("ones1", [1, 128], BF, isOutput=False)
    m01 = nc.declare_dram_parameter("m01", [2, 128, 320], BF, isOutput=False)
    m01b0 = nc.declare_dram_parameter("m01b0", [128, 128], BF, isOutput=False)
    m01b1 = nc.declare_dram_parameter("m01b1", [128, 256], BF, isOutput=False)

    with tile.TileContext(nc) as tc:
        with (
            tc.tile_pool(name="static", bufs=1) as st,
            tc.tile_pool(name="big", bufs=KNOBS["big"]) as bigp,
            tc.tile_pool(name="small", bufs=KNOBS["small"]) as smp,
            tc.tile_pool(name="psS", bufs=KNOBS["psS"], space="PSUM") as psS,
            tc.tile_pool(name="psO", bufs=KNOBS["psO"], space="PSUM") as psO,
            tc.tile_pool(name="psM", bufs=KNOBS["psM"], space="PSUM") as psM,
        ):
            # ---- statics
            # weights stored K-chunked: chunk kc lives at cols [kc*N, (kc+1)*N)
            s_wq = st.tile([128, 2 * D], BF, name="wq", tag="wq")
            s_wk = st.tile([128, 2 * D], BF, name="wk", tag="wk")
            s_wv = st.tile([128, 2 * D], BF, name="wv", tag="wv")
            s_wo = st.tile([128, 2 * D], BF, name="wo", tag="wo")
            s_w1 = st.tile([128, 2 * F], BF, name="w1", tag="w1")
            s_w2 = st.tile([128, 4 * D], BF, name="w2", tag="w2")
            s_dg1 = [st.tile([128, D], BF, name=f"dg1{c}", tag=f"dg1{c}") for c in range(2)]
            s_b1col = st.tile([128, 4], F32, name="b1col", tag="b1col")
            s_browz = st.tile([1, D], BF, name="browz", tag="browz")
            s_g2b = st.tile([128, D], F32, name="g2b", tag="g2b")
            s_b2gb = st.tile([128, D], F32, name="b2gb", tag="b2gb")
            s_ones1 = st.tile([1, 128], BF, name="ones1", tag="ones1")
            s_m01 = [st.tile([128, 320], BF, name=f"m01_{qc}", tag=f"m01_{qc}") for qc in range(2)]
            s_m01b0 = st.tile([128, 128], BF, name="m01b0", tag="m01b0")
            s_eps = st.tile([128, 1], F32, name="eps", tag="eps")
            s_m01b1 = st.tile([128, 256], BF, name="m01b1", tag="m01b1")
            # qk-projection weights first: sample 0's first matmuls need them
            for dst, dsrc, nch_, w_ in [
                (s_wq, wq, 2, D), (s_wk, wk, 2, D), (s_wv, wv, 2, D),
                (s_wo, wo, 2, D), (s_w1, w1, 2, F), (s_w2, w2, 4, D),
            ]:
                for kc in range(nch_):
                    nc.sync.dma_start(out=dst[:, kc * w_:(kc + 1) * w_],
                                      in_=dsrc[kc * 128:(kc + 1) * 128, :])
            for dst, dsrc in [
                (s_m01b0, m01b0), (s_m01b1, m01b1), (s_ones1, ones1),
                (s_b1col, b1col), (s_browz, browz),
                (s_g2b, g2b), (s_b2gb, b2gb),
            ]:
                nc.sync.dma_start(out=dst[:], in_=dsrc[:])
            nc.vector.memset(s_eps[:], EPS)
            for c in range(2):
                nc.sync.dma_start(out=s_dg1[c][:], in_=dg1[c])
                nc.sync.dma_start(out=s_m01[c][:], in_=m01[c])

            # weight chunk kc (K-rows kc*128..) cols [mlo,mhi) of a w_-wide chunk
            def wch(t, kc, w_, mlo, mhi):
                return t[:, kc * w_ + mlo:kc * w_ + mhi]

            # ---- X load / cast / transpose, software-pipelined -------------
            # The load for sample s+1 is issued one sample ahead on the idle
            # gpsimd queue; the cast also runs on gpsimd at the top of its own
            # sample, so neither sits behind busy-queue tails (in-order queues
            # = head-of-line blocking).
            xt_next = {}

            def load_x(s):
                xt_all = bigp.tile([128, 8 * D], F32, name="xt_all", tag="xt_all")
                nc.gpsimd.dma_start(
                    out=xt_all[:],
                    in_=_mkap(xin[s, 0:1, 0:1], 0,
                              [[D, 128], [128 * D, 8], [1, D]]))
                xt_next[s] = xt_all

            load_x(0)
            for s in range(SPC):
                xt_all = xt_next.pop(s)
                xt = [xt_all[:, tc_ * D:(tc_ + 1) * D] for tc_ in range(8)]
                # xb_all col = dc*1024 + tc*128 + p' (C-major, C = dc*8+tc)
                # so ONE batched transpose yields xTall col = C*128 + j
                #   = dc*1024 + tc*128 + j  (the layout consumers expect)
                xb_all = bigp.tile([128, 8 * D], BF, name="xb_all", tag="xb_all")
                nc.gpsimd.tensor_copy(
                    _bcast(xb_all[0:128, 0:1], 0,
                           [[128, 8], [1024, 2], [1, 128]]),
                    _bcast(xt_all[0:128, 0:1], 0,
                           [[256, 8], [128, 2], [1, 128]]))
                if s + 1 < SPC:
                    load_x(s + 1)
                xTall = bigp.tile([128, 2 * L], BF, name="xTall", tag="xTall")
                nc.sync.dma_start_transpose(
                    out=_bcast(xTall[0:128, 0:1], 0, [[128, 16], [1, 128]]),
                    in_=xb_all[:])

                # ---- Q.T / K.T (d-major) and V (token-major) ---------------
                qT = [bigp.tile([128, L], BF, name=f"qT{mc}", tag=f"qT{mc}") for mc in range(2)]
                kT = [bigp.tile([128, L], BF, name=f"kT{mc}", tag=f"kT{mc}") for mc in range(2)]
                for wt, dstl in ((s_wq, qT), (s_wk, kT)):
                    for mc in range(2):
                        for hf in range(2):
                            ps = psM.tile([128, 512], F32, name="mm", tag="mm")
                            for kc in range(2):
                                nc.tensor.matmul(
                                    ps[:], lhsT=wch(wt, kc, D, mc * 128, mc * 128 + 128),
                                    rhs=xTall[:, kc * L + hf * 512:kc * L + (hf + 1) * 512],
                                    start=(kc == 0), stop=(kc == 1))
                            nc.vector.tensor_copy(
                                dstl[mc][:, hf * 512:(hf + 1) * 512], ps[:])
                # head h%4==3 sits at base partition 96 (invalid for matmul
                # operands); extract to offset-0 tiles via SBUF->SBUF DMA
                q3 = [smp.tile([32, L], BF, name=f"q3_{hc}", tag=f"q3_{hc}") for hc in range(2)]
                k3 = [smp.tile([32, L], BF, name=f"k3_{hc}", tag=f"k3_{hc}") for hc in range(2)]
                for hc in range(2):
                    nc.sync.dma_start(out=q3[hc][:], in_=qT[hc][96:128, :])
                    nc.sync.dma_start(out=k3[hc][:], in_=kT[hc][96:128, :])
                # vb: 0-aligned V token-chunks (block0 needs tokens [0,256)).
                # vb64: 64-shifted chunks, vb64[t] rows <-> tokens 64+t*128..,
                # aligning tail-window AV chunks to a single matmul each.
                # Layout [128, 8*33]: head h at cols h*33..h*33+32, col h*33+32
                # is ones -> AV matmul emits softmax denominators for free.
                vb = [bigp.tile([128, 264], BF, name=f"vb{tc_}", tag=f"vb{tc_}") for tc_ in range(2)]
                vb64 = [bigp.tile([128, 264], BF, name=f"vb64_{tc_}", tag=f"vb64_{tc_}") for tc_ in range(8)]
                for tc_ in range(2):
                    ps = psM.tile([128, D], F32, name="mm", tag="mm")
                    for kc in range(2):
                        nc.tensor.matmul(
                            ps[:], lhsT=xTall[:, kc * L + tc_ * 128:kc * L + (tc_ + 1) * 128],
                            rhs=s_wv[:, kc * D:(kc + 1) * D],
                            start=(kc == 0), stop=(kc == 1))
                    nc.vector.tensor_copy(
                        _bcast(vb[tc_][0:128, 0:1], 0, [[33, 8], [1, 32]]),
                        ps[:])
                    nc.gpsimd.memset(
                        _bcast(vb[tc_][0:128, 0:1], 32, [[33, 8]]), 1.0)
                for tc_ in range(8):
                    rows = 128 if tc_ < 7 else 64
                    ps = psM.tile([128, D], F32, name="mm", tag="mm")
                    for kc in range(2):
                        nc.tensor.matmul(
                            ps[:rows, :],
                            lhsT=xTall[:, kc * L + 64 + tc_ * 128:kc * L + 64 + tc_ * 128 + rows],
                            rhs=s_wv[:, kc * D:(kc + 1) * D],
                            start=(kc == 0), stop=(kc == 1))
                    nc.vector.tensor_copy(
                        _bcast(vb64[tc_][0:rows, 0:1], 0, [[33, 8], [1, 32]]),
                        ps[:rows, :])
                    nc.gpsimd.memset(
                        _bcast(vb64[tc_][0:rows, 0:1], 32, [[33, 8]]), 1.0)

                # ---- attention --------------------------------------------
                # Oall col = dc*1024 + tc*128 + (d - dc*128) so ONE batched
                # transpose produces OTall (see xb_all comment).
                Oall = bigp.tile([128, 8 * D], BF, name="Oall", tag="Oall")
                for b in range(NB):
                    for qc in range(2):
                        lo, hi = _win(b, qc)
                        w = hi - lo
                        wpad = -(-w // 128) * 128
                        nch = wpad // 128
                        qlo = b * 256 + qc * 128
                        # P head-major, 384-stride: col h*384 + j (key j of
                        # head h; j < 320 valid, [320,384) garbage whose
                        # transposed rows are excluded by vr=64). Contiguous
                        # in j -> ONE DVE mask-mult; one PT transpose per bq.
                        Pall = bigp.tile([128, 8 * 384], BF, name="Pall", tag="Pall")
                        PT = bigp.tile([128, 8 * 384], BF, name="PT", tag="PT")
                        Ou = psO.tile([128, 264], F32, name="Ou", tag="Ou")
                        Praw = bigp.tile([128, 8 * 320], BF, name="Praw", tag="Praw")
                        for h in range(H):
                            hc, hr = h // 4, (h % 4) * 32
                            if h % 4 == 3:
                                qsl = q3[hc][0:32, qlo:qlo + 128]
                                ksl = k3[hc][0:32, lo:hi]
                            else:
                                qsl = qT[hc][hr:hr + 32, qlo:qlo + 128]
                                ksl = kT[hc][hr:hr + 32, lo:hi]
                            S = psS.tile([128, 320], F32, name="S", tag="S")
                            # raw QK (no mask inject), exp, then mult by a
                            # 0/1 mask (masked entries exp(S)*0 = 0)
                            nc.tensor.matmul(
                                S[:, :w], lhsT=qsl, rhs=ksl,
                                start=True, stop=True)
                            nc.scalar.activation(
                                out=Praw[:, h * 320:h * 320 + w],
                                in_=S[:, :w], func=AF.Exp)
                        if b == 0:
                            # b0: chunk-major Pall (col ci*1024 + h*128 + j)
                            # so the transpose only moves nch*1024 cols
                            m01a = s_m01b0 if qc == 0 else s_m01b1
                            nc.vector.tensor_tensor(
                                out=_bcast(Pall[0:128, 0:1], 0,
                                           [[128, 8], [1024, nch], [1, 128]]),
                                in0=_bcast(Praw[0:128, 0:1], 0,
                                           [[320, 8], [128, nch], [1, 128]]),
                                in1=_bcast(m01a[0:128, 0:1], 0,
                                           [[0, 8], [128, nch], [1, 128]]),
                                op=OP.mult)
                        else:
                            nc.vector.tensor_tensor(
                                out=_bcast(Pall[0:128, 0:1], 0,
                                           [[384, 8], [1, 320]]),
                                in0=_bcast(Praw[0:128, 0:1], 0,
                                           [[320, 8], [1, 320]]),
                                in1=_bcast(s_m01[qc][0:128, 0:1], 0,
                                           [[0, 8], [1, 320]]),
                                op=OP.mult)
                        if b == 0:
                            nc.sync.dma_start_transpose(
                                out=_bcast(PT[0:128, 0:1], 0,
                                           [[128, 8 * nch], [1, 128]]),
                                in_=Pall[:, 0:nch * 1024])
                        else:
                            nc.sync.dma_start_transpose(
                                out=_bcast(PT[0:128, 0:1], 0,
                                           [[128, 24], [1, 128]]),
                                in_=Pall[:])
                        for h in range(H):
                            for ci in range(nch):
                                vr = min(128, w - ci * 128)
                                if b == 0:
                                    vt = vb[ci]
                                    blk = ci * 8 + h
                                else:
                                    vt = vb64[(lo + ci * 128 - 64) // 128]
                                    blk = h * 3 + ci
                                nc.tensor.matmul(
                                    Ou[:, h * 33:(h + 1) * 33],
                                    lhsT=PT[0:vr, blk * 128:(blk + 1) * 128],
                                    rhs=vt[0:vr, h * 33:(h + 1) * 33],
                                    start=(ci == 0), stop=(ci == nch - 1))
                        rec = smp.tile([128, 8], F32, name="rec", tag="rec")
                        nc.vector.reciprocal(
                            rec[:], _bcast(Ou[0:128, 0:1], 32, [[33, 8]]))
                        tc_o = 2 * b + qc
                        nc.vector.tensor_tensor(
                            out=_bcast(Oall[0:128, 0:1], tc_o * 128,
                                       [[1024, 2], [32, 4], [1, 32]]),
                            in0=_bcast(Ou[0:128, 0:1], 0,
                                       [[132, 2], [33, 4], [1, 32]]),
                            in1=_bcast(rec[0:128, 0:1], 0,
                                       [[4, 2], [1, 4], [0, 32]]),
                            op=OP.mult)

                OTall = bigp.tile([128, 2 * L], BF, name="OTall", tag="OTall")
                nc.scalar.dma_start_transpose(
                    out=_bcast(OTall[0:128, 0:1], 0, [[128, 16], [1, 128]]),
                    in_=Oall[:])

                # ---- Wo + residual + LN1 ----------------------------------
                z1n_all = bigp.tile([128, 8 * D], BF, name="z1n_all", tag="z1n_all")
                for tc_ in range(8):
                    ps = psM.tile([128, D], F32, name="mm", tag="mm")
                    for dc in range(2):
                        nc.tensor.matmul(
                            ps[:], lhsT=OTall[:, dc * L + tc_ * 128:dc * L + (tc_ + 1) * 128],
                            rhs=s_wo[:, dc * D:(dc + 1) * D],
                            start=(dc == 0), stop=(dc == 1))
                    z1 = smp.tile([128, D], F32, name="z1", tag="z1")
                    nc.vector.tensor_tensor(out=z1[:], in0=ps[:], in1=xt[tc_][:],
                                            op=OP.add)
                    st6 = smp.tile([128, 6], F32, name="st6", tag="st6")
                    st2 = smp.tile([128, 2], F32, name="st2", tag="st2")
                    nc.vector.bn_stats(st6[:], z1[:])
                    nc.vector.bn_aggr(st2[:], st6[:])
                    std = smp.tile([128, 1], F32, name="std", tag="std")
                    nc.scalar.activation(out=std[:], in_=st2[:, 1:2], func=AF.Sqrt,
                                         bias=s_eps[:, 0:1])
                    rstd = smp.tile([128, 1], F32, name="rstd", tag="rstd")
                    nc.vector.reciprocal(rstd[:], std[:])
                    nc.vector.tensor_scalar(
                        out=_bcast(z1n_all[0:128, 0:1], tc_ * 128,
                                   [[1024, 2], [1, 128]]),
                        in0=z1[:],
                        scalar1=st2[:, 0:1],
                        scalar2=rstd[:], op0=OP.subtract, op1=OP.mult)

                z1nTall = bigp.tile([128, 2 * L], BF, name="z1nTall", tag="z1nTall")
                nc.scalar.dma_start_transpose(
                    out=_bcast(z1nTall[0:128, 0:1], 0, [[128, 16], [1, 128]]),
                    in_=z1n_all[:])

                # ---- MLP up (h.T orientation) + fused bias+relu -----------
                # h.T = W1eff.T-chunks @ z1nT; relu(x + b1) with b1 per-
                # partition in this orientation -> no r transpose needed.
                rTall = bigp.tile([128, 4 * L], BF, name="rTall", tag="rTall")
                for fc in range(4):
                    for hf in range(2):
                        ps = psM.tile([128, 512], F32, name="mm", tag="mm")
                        for dc in range(2):
                            nc.tensor.matmul(
                                ps[:],
                                lhsT=s_w1[:, dc * F + fc * 128:dc * F + (fc + 1) * 128],
                                rhs=z1nTall[:, dc * L + hf * 512:dc * L + (hf + 1) * 512],
                                start=(dc == 0), stop=(dc == 1))
                        dst = rTall[:, fc * L + hf * 512:fc * L + (hf + 1) * 512]
                        if (fc + hf) % 2 == 0:
                            nc.scalar.activation(out=dst, in_=ps[:], func=AF.Relu,
                                                 bias=s_b1col[:, fc:fc + 1])
                        else:
                            nc.vector.tensor_scalar(
                                out=dst, in0=ps[:], scalar1=s_b1col[:, fc:fc + 1],
                                scalar2=0.0, op0=OP.add, op1=OP.max)

                # ---- MLP down + injects + LN2 + out -----------------------
                ot_all = bigp.tile([128, 8 * D], F32, name="ot_all", tag="ot_all")
                for tc_ in range(8):
                    ps = psM.tile([128, D], F32, name="mm", tag="mm")
                    for fc in range(4):
                        nc.tensor.matmul(
                            ps[:], lhsT=rTall[:, fc * L + tc_ * 128:fc * L + (tc_ + 1) * 128],
                            rhs=s_w2[:, fc * D:(fc + 1) * D],
                            start=(fc == 0), stop=False)
                    for dc in range(2):
                        nc.tensor.matmul(
                            ps[:], lhsT=z1nTall[:, dc * L + tc_ * 128:dc * L + (tc_ + 1) * 128],
                            rhs=s_dg1[dc][:], start=False, stop=False)
                    nc.tensor.matmul(ps[:], lhsT=s_ones1[:], rhs=s_browz[:],
                                     start=False, stop=True)
                    st6 = smp.tile([128, 6], F32, name="st6", tag="st6")
                    st2 = smp.tile([128, 2], F32, name="st2", tag="st2")
                    nc.vector.bn_stats(st6[:], ps[:])
                    nc.vector.bn_aggr(st2[:], st6[:])
                    std = smp.tile([128, 1], F32, name="std", tag="std")
                    nc.scalar.activation(out=std[:], in_=st2[:, 1:2], func=AF.Sqrt,
                                         bias=s_eps[:, 0:1])
                    rstd = smp.tile([128, 1], F32, name="rstd", tag="rstd")
                    nc.vector.reciprocal(rstd[:], std[:])
                    t1 = smp.tile([128, D], F32, name="t1", tag="t1")
                    nc.vector.scalar_tensor_tensor(
                        out=t1[:], in0=ps[:], scalar=st2[:, 0:1], in1=s_g2b[:],
                        op0=OP.subtract, op1=OP.mult)
                    nc.vector.scalar_tensor_tensor(
                        out=ot_all[:, tc_ * D:(tc_ + 1) * D], in0=t1[:],
                        scalar=rstd[:], in1=s_b2gb[:],
                        op0=OP.mult, op1=OP.add)
                nc.sync.dma_start(
                    out=_mkap(out[s, 0:1, 0:1], 0,
                              [[D, 128], [128 * D, 8], [1, D]]),
                    in_=ot_all[:])

    _split_sync_waits(nc)
    _CACHE["nc"] = nc
    return nc


def _in_maps(X, Wq, Wk, Wv, Wo, ln1_g, ln1_b, W1, b1, W2, b2, ln2_g, ln2_b):
    X = np.asarray(X, dtype=np.float32)
    f32 = lambda a: np.asarray(a, dtype=np.float32)
    Wq, Wk, Wv, Wo = f32(Wq), f32(Wk), f32(Wv), f32(Wo)
    W1, W2 = f32(W1), f32(W2)
    ln1_g, ln1_b, b1, b2 = f32(ln1_g), f32(ln1_b), f32(b1), f32(b2)
    ln2_g, ln2_b = f32(ln2_g), f32(ln2_b)

    bf = ml_dtypes.bfloat16
    w1eff = (ln1_g[:, None] * W1)
    b1eff = (b1 + ln1_b @ W1)
    dg1 = np.zeros((2, 128, D), np.float32)
    for c in range(2):
        for i in range(128):
            dg1[c, i, c * 128 + i] = ln1_g[c * 128 + i]
    m2d = _mask2d()
    mt_core = np.stack([m2d[0:128, 64:384], m2d[128:256, 192:512]])
    m01v = (mt_core == 0.0).astype(np.float32)  # [2, 128, 320] 0/1
    cm = _causal_mask()
    statics = {
        "wq": (Wq * (DH ** -0.5)).astype(bf),
        "wk": Wk.astype(bf),
        "wv": Wv.astype(bf),
        "wo": Wo.astype(bf),
        "w1": w1eff.astype(bf),
        "w2": W2.astype(bf),
        "dg1": dg1.astype(bf),
        "b1col": b1eff.reshape(4, 128).T.astype(np.float32).copy(),
        "browz": (ln1_b + b2)[None, :].astype(bf),
        "g2b": np.tile(ln2_g[None, :], (128, 1)).astype(np.float32),
        "b2gb": np.tile(ln2_b[None, :], (128, 1)).astype(np.float32),
        "ones1": np.ones((1, 128)).astype(bf),
        "m01": m01v.astype(bf),
        "m01b0": (cm[0:128, 0:128] == 0.0).astype(np.float32).astype(bf),
        "m01b1": (cm[128:256, 0:256] == 0.0).astype(np.float32).astype(bf),
    }

    in_maps = []
    for i in range(N_CORES):
        m = {"X": X[i * SPC:(i + 1) * SPC]}
        m.update(statics)
        in_maps.append(m)
    return in_maps


def kernel(**inputs):
    from concourse.bass_utils import run_bass_kernel_spmd
    nc = _build()
    res = run_bass_kernel_spmd(nc, _in_maps(**inputs), list(range(N_CORES)))
    return np.concatenate([res.results[i]["OUT"] for i in range(N_CORES)], axis=0)


def kernel_profiled(tmpdir=None, **inputs):
    from concourse.bass_utils import run_bass_kernel_spmd
    nc = _build()
    res = run_bass_kernel_spmd(nc, _in_maps(**inputs), list(range(N_CORES)),
                               trace=True, tmpdir=tmpdir)
    out = np.concatenate([res.results[i]["OUT"] for i in range(N_CORES)], axis=0)
    return out, res



# revision 28
# speedup vs baseline: 1.1436x; 1.0037x over previous
"""Trainium2 Bass kernel for nn_DecoderLayer_11424613007924.

DecoderLayer: block-sparse attention (BLEN=256, causal first block,
2D-local windowed tail blocks) + LayerNorm + MLP, fp32 I/O.

Sharding: data-parallel over batch. 32 samples -> 8 NeuronCores x 4.

Per-core dataflow (per sample):
  X [1024,256] --(cast bf16, DMA-transpose)--> X.T
  Q.T/K.T = W.T @ X.T (PE, bf16);  V = X.T-stationary @ Wv (token-major)
  per (block b, query-chunk qc, head h):
      S  = q.T-chunk.T @ k.T-window (+ mask via identity-matmul inject, PSUM)
      P  = exp(S)  (ACT, accum_out -> softmax denominators)
      P.T via DMA-transpose; O_u = P.T.T @ v (PE); O = O_u * recip(den) (DVE)
  y = O.T-stationary @ Wo; z1 = y + X (fp32)
  LN1 via bn_stats; z1n = (z1-m)*rstd (g1 folded into W1 host-side)
  h = z1n.T @ W1eff + b1eff (ones-row inject); r = relu(h)
  z2 = r.T @ W2 + (ln1_b+b2) inject + z1n*g1 inject (diag(g1) matmul)
  out = (z2-m2)*g2b*rstd2 + b2gb  (scalar_tensor_tensor x2)

Numerics: matmul operands bf16 (PSUM fp32 accum); residual stream,
LN stats, softmax denominators fp32. Masks use -30 instead of -1e9
(exp(-30)*512 ~ 5e-11 relative contamination).
"""
import numpy as np
import ml_dtypes

import concourse.bass as bass
import concourse.mybir as mybir
import concourse.tile as tile
from bass_rust import ScopedClock

BF = mybir.dt.bfloat16
F32 = mybir.dt.float32
AF = mybir.ActivationFunctionType
OP = mybir.AluOpType

N_CORES = 8
B, L, D = 32, 1024, 256
H, DH, F = 8, 32, 512
BLEN = 256
SPC = B // N_CORES  # samples per core
NB = L // BLEN      # 4 blocks
NEG = -30.0
EPS = 1e-6

# ---------------------------------------------------------------- fixups ---
# This container's walrus build rejects instructions carrying >1 semaphore
# wait. Split extra waits onto same-engine NOPs after Tile scheduling.


def _split_sync_waits(nc):
    for fn in nc.m.functions:
        for bb in fn.blocks:
            insts = bb.instructions
            if not insts:
                continue
            new_list = []
            n_split = 0
            for inst in insts:
                si = inst.sync_info
                waits = list(si.on_wait) if (si and si.on_wait) else []
                if len(waits) > 1:
                    si.on_wait = waits[:1]
                    for w in waits[1:]:
                        nop = nc.engines[inst.engine].nop()
                        for f2 in nc.m.functions:
                            for b2 in f2.blocks:
                                l2 = b2.instructions
                                if l2 and l2[-1] is nop.ins:
                                    l2.pop()
                        nop.ins.sync_info = mybir.SyncInfo(on_wait=[w], on_update=[])
                        new_list.append(nop.ins)
                        n_split += 1
                new_list.append(inst)
            if n_split:
                bb.instructions = new_list


def _patched_drain_and_barrier(self, tick_clock, wait_clock):
    nc = self.nc
    probe = nc.sync.nop()
    wait_clock.add_sem_waits(probe.ins, ScopedClock({None: tick_clock.global_clock}))
    nc.sync.drain()
    nc.all_engine_barrier()
    assert self.sems is not None
    popped = nc._tile_sem_poison_stack.pop()
    assert popped is self._sem_poison
    nc.clear_and_free_semaphores(list(self.sems.allocated().values()))
    nc.all_engine_barrier()


tile.TileContext._drain_and_barrier = _patched_drain_and_barrier


# ------------------------------------------------------------- host prep ---

def _mask2d(blen=BLEN, h=32, win=6):
    cp = np.arange(blen, 2 * blen)[:, None]
    op = np.arange(2 * blen)[None, :]
    causal = op <= cp
    ch, cw = cp // h, cp % h
    oh, ow = op // h, op % h
    ok = causal & (np.abs(ch - oh) <= win) & (np.abs(cw - ow) <= win)
    return np.where(ok, 0.0, NEG).astype(np.float32)


def _causal_mask(blen=BLEN):
    return np.where(np.tril(np.ones((blen, blen), bool)), 0.0, NEG).astype(np.float32)


def _bcast(ap, offset_extra, plist):
    """AP with explicit [step,count] free dims appended to partition dim."""
    return bass.AP(tensor=ap.tensor, offset=ap.offset + offset_extra,
                   ap=[list(ap.ap[0])] + plist)


def _mkap(ap, offset_extra, dims):
    """AP with fully explicit [step,count] dims (incl. partition dim)."""
    return bass.AP(tensor=ap.tensor, offset=ap.offset + offset_extra, ap=dims)


def _win(b, qc):
    """Key window (absolute token range) for block b, query chunk qc."""
    if b == 0:
        return 0, 128 * (qc + 1)
    a = (b - 1) * 256 + 64 + 128 * qc
    return a, a + 320




_CACHE = {}

# tuning knobs (read at build time)
KNOBS = {"psS": 3, "psO": 2, "psM": 3, "big": 2, "small": 2}


def _build():
    if "nc" in _CACHE:
        return _CACHE["nc"]
    nc = bass.Bass(target_bir_lowering=False)

    xin = nc.declare_dram_parameter("X", [SPC, L, D], F32, isOutput=False)
    out = nc.declare_dram_parameter("OUT", [SPC, L, D], F32, isOutput=True)
    wq = nc.declare_dram_parameter("wq", [D, D], BF, isOutput=False)
    wk = nc.declare_dram_parameter("wk", [D, D], BF, isOutput=False)
    wv = nc.declare_dram_parameter("wv", [D, D], BF, isOutput=False)
    wo = nc.declare_dram_parameter("wo", [D, D], BF, isOutput=False)
    w1 = nc.declare_dram_parameter("w1", [D, F], BF, isOutput=False)
    w2 = nc.declare_dram_parameter("w2", [F, D], BF, isOutput=False)
    dg1 = nc.declare_dram_parameter("dg1", [2, 128, D], BF, isOutput=False)
    b1col = nc.declare_dram_parameter("b1col", [128, 4], F32, isOutput=False)
    browz = nc.declare_dram_parameter("browz", [1, D], BF, isOutput=False)
    g2b = nc.declare_dram_parameter("g2b", [128, D], F32, isOutput=False)
    b2gb = nc.declare_dram_parameter("b2gb", [128, D], F32, isOutput=False)
    ones1 = nc.declare_dram_parameter("ones1", [1, 128], BF, isOutput=False)
    m01 = nc.declare_dram_parameter("m01", [2, 128, 320], BF, isOutput=False)
    m01b0 = nc.declare_dram_parameter("m01b0", [128, 128], BF, isOutput=False)
    m01b1 = nc.declare_dram_parameter("m01b1", [128, 256], BF, isOutput=False)

    with tile.TileContext(nc) as tc:
        with (
            tc.tile_pool(name="static", bufs=1) as st,
            tc.tile_pool(name="big", bufs=KNOBS["big"]) as bigp,
            tc.tile_pool(name="small", bufs=KNOBS["small"]) as smp,
            tc.tile_pool(name="psS", bufs=KNOBS["psS"], space="PSUM") as psS,
            tc.tile_pool(name="psO", bufs=KNOBS["psO"], space="PSUM") as psO,
            tc.tile_pool(name="psM", bufs=KNOBS["psM"], space="PSUM") as psM,
        ):
            # ---- statics
            # weights stored K-chunked: chunk kc lives at cols [kc*N, (kc+1)*N)
            s_wq = st.tile([128, 2 * D], BF, name="wq", tag="wq")
            s_wk = st.tile([128, 2 * D], BF, name="wk", tag="wk")
            s_wv = st.tile([128, 2 * D], BF, name="wv", tag="wv")
            s_wo = st.tile([128, 2 * D], BF, name="wo", tag="wo")
            s_w1 = st.tile([128, 2 * F], BF, name="w1", tag="w1")
            s_w2 = st.tile([128, 4 * D], BF, name="w2", tag="w2")
            s_dg1 = [st.tile([128, D], BF, name=f"dg1{c}", tag=f"dg1{c}") for c in range(2)]
            s_b1col = st.tile([128, 4], F32, name="b1col", tag="b1col")
            s_browz = st.tile([1, D], BF, name="browz", tag="browz")
            s_g2b = st.tile([128, D], F32, name="g2b", tag="g2b")
            s_b2gb = st.tile([128, D], F32, name="b2gb", tag="b2gb")
            s_ones1 = st.tile([1, 128], BF, name="ones1", tag="ones1")
            s_m01 = [st.tile([128, 320], BF, name=f"m01_{qc}", tag=f"m01_{qc}") for qc in range(2)]
            s_m01b0 = st.tile([128, 128], BF, name="m01b0", tag="m01b0")
            s_eps = st.tile([128, 1], F32, name="eps", tag="eps")
            s_m01b1 = st.tile([128, 256], BF, name="m01b1", tag="m01b1")
            # qk-projection weights first: sample 0's first matmuls need them
            # statics on the scalar queue: sync stays free for sample 0's
            # X transpose, so compute can start while statics stream in
            for dst, dsrc, nch_, w_ in [
                (s_wq, wq, 2, D), (s_wk, wk, 2, D), (s_wv, wv, 2, D),
                (s_wo, wo, 2, D), (s_w1, w1, 2, F), (s_w2, w2, 4, D),
            ]:
                for kc in range(nch_):
                    nc.scalar.dma_start(out=dst[:, kc * w_:(kc + 1) * w_],
                                        in_=dsrc[kc * 128:(kc + 1) * 128, :])
            for dst, dsrc in [
                (s_m01b0, m01b0), (s_m01b1, m01b1), (s_ones1, ones1),
                (s_b1col, b1col), (s_browz, browz),
                (s_g2b, g2b), (s_b2gb, b2gb),
            ]:
                nc.scalar.dma_start(out=dst[:], in_=dsrc[:])
            nc.vector.memset(s_eps[:], EPS)
            for c in range(2):
                nc.scalar.dma_start(out=s_dg1[c][:], in_=dg1[c])
                nc.scalar.dma_start(out=s_m01[c][:], in_=m01[c])

            # weight chunk kc (K-rows kc*128..) cols [mlo,mhi) of a w_-wide chunk
            def wch(t, kc, w_, mlo, mhi):
                return t[:, kc * w_ + mlo:kc * w_ + mhi]

            # ---- X load / cast / transpose, software-pipelined -------------
            # load runs 2 samples ahead (gpsimd queue), cast 1 ahead (DVE,
            # input long-ready so no head-of-line stall), and the transpose
            # for s+1 is issued on sync just before s's store. By the time
            # sample s+1 starts, xTall(s+1) is already in SBUF.
            xt_next, xb_next, xT_next = {}, {}, {}

            def load_x(s):
                xt_all = bigp.tile([128, 8 * D], F32, name="xt_all",
                                   tag="xt_all", bufs=3)
                nc.gpsimd.dma_start(
                    out=xt_all[:],
                    in_=_mkap(xin[s, 0:1, 0:1], 0,
                              [[D, 128], [128 * D, 8], [1, D]]))
                xt_next[s] = xt_all

            def cast_x(s):
                # xb_all col = dc*1024 + tc*128 + p' (C-major, C = dc*8+tc)
                # so ONE batched transpose yields xTall col = C*128 + j
                #   = dc*1024 + tc*128 + j  (the layout consumers expect)
                xb_all = bigp.tile([128, 8 * D], BF, name="xb_all", tag="xb_all")
                nc.vector.tensor_copy(
                    _bcast(xb_all[0:128, 0:1], 0,
                           [[128, 8], [1024, 2], [1, 128]]),
                    _bcast(xt_next[s][0:128, 0:1], 0,
                           [[256, 8], [128, 2], [1, 128]]))
                xb_next[s] = xb_all

            def xpose_x(s):
                xTall = bigp.tile([128, 2 * L], BF, name="xTall", tag="xTall")
                nc.sync.dma_start_transpose(
                    out=_bcast(xTall[0:128, 0:1], 0, [[128, 16], [1, 128]]),
                    in_=xb_next.pop(s)[:])
                xT_next[s] = xTall

            load_x(0)
            if SPC > 1:
                load_x(1)
            cast_x(0)
            xpose_x(0)
            for s in range(SPC):
                if s + 2 < SPC:
                    load_x(s + 2)
                if s + 1 < SPC:
                    cast_x(s + 1)
                xt_all = xt_next.pop(s)
                xt = [xt_all[:, tc_ * D:(tc_ + 1) * D] for tc_ in range(8)]
                xTall = xT_next.pop(s)

                # ---- Q.T / K.T (d-major) and V (token-major) ---------------
                qT = [bigp.tile([128, L], BF, name=f"qT{mc}", tag=f"qT{mc}") for mc in range(2)]
                kT = [bigp.tile([128, L], BF, name=f"kT{mc}", tag=f"kT{mc}") for mc in range(2)]
                for wt, dstl in ((s_wq, qT), (s_wk, kT)):
                    for mc in range(2):
                        for hf in range(2):
                            ps = psM.tile([128, 512], F32, name="mm", tag="mm")
                            for kc in range(2):
                                nc.tensor.matmul(
                                    ps[:], lhsT=wch(wt, kc, D, mc * 128, mc * 128 + 128),
                                    rhs=xTall[:, kc * L + hf * 512:kc * L + (hf + 1) * 512],
                                    start=(kc == 0), stop=(kc == 1))
                            nc.vector.tensor_copy(
                                dstl[mc][:, hf * 512:(hf + 1) * 512], ps[:])
                # head h%4==3 sits at base partition 96 (invalid for matmul
                # operands); extract to offset-0 tiles via SBUF->SBUF DMA
                q3 = [smp.tile([32, L], BF, name=f"q3_{hc}", tag=f"q3_{hc}") for hc in range(2)]
                k3 = [smp.tile([32, L], BF, name=f"k3_{hc}", tag=f"k3_{hc}") for hc in range(2)]
                for hc in range(2):
                    nc.sync.dma_start(out=q3[hc][:], in_=qT[hc][96:128, :])
                    nc.sync.dma_start(out=k3[hc][:], in_=kT[hc][96:128, :])
                # vb: 0-aligned V token-chunks (block0 needs tokens [0,256)).
                # vb64: 64-shifted chunks, vb64[t] rows <-> tokens 64+t*128..,
                # aligning tail-window AV chunks to a single matmul each.
                # Layout [128, 8*33]: head h at cols h*33..h*33+32, col h*33+32
                # is ones -> AV matmul emits softmax denominators for free.
                vb = [bigp.tile([128, 264], BF, name=f"vb{tc_}", tag=f"vb{tc_}") for tc_ in range(2)]
                vb64 = [bigp.tile([128, 264], BF, name=f"vb64_{tc_}", tag=f"vb64_{tc_}") for tc_ in range(8)]
                for tc_ in range(2):
                    ps = psM.tile([128, D], F32, name="mm", tag="mm")
                    for kc in range(2):
                        nc.tensor.matmul(
                            ps[:], lhsT=xTall[:, kc * L + tc_ * 128:kc * L + (tc_ + 1) * 128],
                            rhs=s_wv[:, kc * D:(kc + 1) * D],
                            start=(kc == 0), stop=(kc == 1))
                    nc.vector.tensor_copy(
                        _bcast(vb[tc_][0:128, 0:1], 0, [[33, 8], [1, 32]]),
                        ps[:])
                    nc.gpsimd.memset(
                        _bcast(vb[tc_][0:128, 0:1], 32, [[33, 8]]), 1.0)
                for tc_ in range(8):
                    rows = 128 if tc_ < 7 else 64
                    ps = psM.tile([128, D], F32, name="mm", tag="mm")
                    for kc in range(2):
                        nc.tensor.matmul(
                            ps[:rows, :],
                            lhsT=xTall[:, kc * L + 64 + tc_ * 128:kc * L + 64 + tc_ * 128 + rows],
                            rhs=s_wv[:, kc * D:(kc + 1) * D],
                            start=(kc == 0), stop=(kc == 1))
                    nc.vector.tensor_copy(
                        _bcast(vb64[tc_][0:rows, 0:1], 0, [[33, 8], [1, 32]]),
                        ps[:rows, :])
                    nc.gpsimd.memset(
                        _bcast(vb64[tc_][0:rows, 0:1], 32, [[33, 8]]), 1.0)

                # ---- attention --------------------------------------------
                # Oall col = dc*1024 + tc*128 + (d - dc*128) so ONE batched
                # transpose produces OTall (see xb_all comment).
                Oall = bigp.tile([128, 8 * D], BF, name="Oall", tag="Oall")
                for b in range(NB):
                    for qc in range(2):
                        lo, hi = _win(b, qc)
                        w = hi - lo
                        wpad = -(-w // 128) * 128
                        nch = wpad // 128
                        qlo = b * 256 + qc * 128
                        # P head-major, 384-stride: col h*384 + j (key j of
                        # head h; j < 320 valid, [320,384) garbage whose
                        # transposed rows are excluded by vr=64). Contiguous
                        # in j -> ONE DVE mask-mult; one PT transpose per bq.
                        Pall = bigp.tile([128, 8 * 384], BF, name="Pall", tag="Pall")
                        PT = bigp.tile([128, 8 * 384], BF, name="PT", tag="PT")
                        Ou = psO.tile([128, 264], F32, name="Ou", tag="Ou")
                        Praw = bigp.tile([128, 8 * 320], BF, name="Praw", tag="Praw")
                        for h in range(H):
                            hc, hr = h // 4, (h % 4) * 32
                            if h % 4 == 3:
                                qsl = q3[hc][0:32, qlo:qlo + 128]
                                ksl = k3[hc][0:32, lo:hi]
                            else:
                                qsl = qT[hc][hr:hr + 32, qlo:qlo + 128]
                                ksl = kT[hc][hr:hr + 32, lo:hi]
                            S = psS.tile([128, 320], F32, name="S", tag="S")
                            # raw QK (no mask inject), exp, then mult by a
                            # 0/1 mask (masked entries exp(S)*0 = 0)
                            nc.tensor.matmul(
                                S[:, :w], lhsT=qsl, rhs=ksl,
                                start=True, stop=True)
                            nc.scalar.activation(
                                out=Praw[:, h * 320:h * 320 + w],
                                in_=S[:, :w], func=AF.Exp)
                        if b == 0:
                            # b0: chunk-major Pall (col ci*1024 + h*128 + j)
                            # so the transpose only moves nch*1024 cols
                            m01a = s_m01b0 if qc == 0 else s_m01b1
                            nc.vector.tensor_tensor(
                                out=_bcast(Pall[0:128, 0:1], 0,
                                           [[128, 8], [1024, nch], [1, 128]]),
                                in0=_bcast(Praw[0:128, 0:1], 0,
                                           [[320, 8], [128, nch], [1, 128]]),
                                in1=_bcast(m01a[0:128, 0:1], 0,
                                           [[0, 8], [128, nch], [1, 128]]),
                                op=OP.mult)
                        else:
                            nc.vector.tensor_tensor(
                                out=_bcast(Pall[0:128, 0:1], 0,
                                           [[384, 8], [1, 320]]),
                                in0=_bcast(Praw[0:128, 0:1], 0,
                                           [[320, 8], [1, 320]]),
                                in1=_bcast(s_m01[qc][0:128, 0:1], 0,
                                           [[0, 8], [1, 320]]),
                                op=OP.mult)
                        if b == 0:
                            nc.sync.dma_start_transpose(
                                out=_bcast(PT[0:128, 0:1], 0,
                                           [[128, 8 * nch], [1, 128]]),
                                in_=Pall[:, 0:nch * 1024])
                        else:
                            nc.sync.dma_start_transpose(
                                out=_bcast(PT[0:128, 0:1], 0,
                                           [[128, 24], [1, 128]]),
                                in_=Pall[:])
                        for h in range(H):
                            for ci in range(nch):
                                vr = min(128, w - ci * 128)
                                if b == 0:
                                    vt = vb[ci]
                                    blk = ci * 8 + h
                                else:
                                    vt = vb64[(lo + ci * 128 - 64) // 128]
                                    blk = h * 3 + ci
                                nc.tensor.matmul(
                                    Ou[:, h * 33:(h + 1) * 33],
                                    lhsT=PT[0:vr, blk * 128:(blk + 1) * 128],
                                    rhs=vt[0:vr, h * 33:(h + 1) * 33],
                                    start=(ci == 0), stop=(ci == nch - 1))
                        rec = smp.tile([128, 8], F32, name="rec", tag="rec")
                        nc.vector.reciprocal(
                            rec[:], _bcast(Ou[0:128, 0:1], 32, [[33, 8]]))
                        tc_o = 2 * b + qc
                        nc.vector.tensor_tensor(
                            out=_bcast(Oall[0:128, 0:1], tc_o * 128,
                                       [[1024, 2], [32, 4], [1, 32]]),
                            in0=_bcast(Ou[0:128, 0:1], 0,
                                       [[132, 2], [33, 4], [1, 32]]),
                            in1=_bcast(rec[0:128, 0:1], 0,
                                       [[4, 2], [1, 4], [0, 32]]),
                            op=OP.mult)

                OTall = bigp.tile([128, 2 * L], BF, name="OTall", tag="OTall")
                nc.sync.dma_start_transpose(
                    out=_bcast(OTall[0:128, 0:1], 0, [[128, 16], [1, 128]]),
                    in_=Oall[:])

                # ---- Wo + residual + LN1 ----------------------------------
                z1n_all = bigp.tile([128, 8 * D], BF, name="z1n_all", tag="z1n_all")
                for tc_ in range(8):
                    ps = psM.tile([128, D], F32, name="mm", tag="mm")
                    for dc in range(2):
                        nc.tensor.matmul(
                            ps[:], lhsT=OTall[:, dc * L + tc_ * 128:dc * L + (tc_ + 1) * 128],
                            rhs=s_wo[:, dc * D:(dc + 1) * D],
                            start=(dc == 0), stop=(dc == 1))
                    z1 = smp.tile([128, D], F32, name="z1", tag="z1")
                    nc.vector.tensor_tensor(out=z1[:], in0=ps[:], in1=xt[tc_][:],
                                            op=OP.add)
                    st6 = smp.tile([128, 6], F32, name="st6", tag="st6")
                    st2 = smp.tile([128, 2], F32, name="st2", tag="st2")
                    nc.vector.bn_stats(st6[:], z1[:])
                    nc.vector.bn_aggr(st2[:], st6[:])
                    std = smp.tile([128, 1], F32, name="std", tag="std")
                    nc.scalar.activation(out=std[:], in_=st2[:, 1:2], func=AF.Sqrt,
                                         bias=s_eps[:, 0:1])
                    rstd = smp.tile([128, 1], F32, name="rstd", tag="rstd")
                    nc.vector.reciprocal(rstd[:], std[:])
                    nc.vector.tensor_scalar(
                        out=_bcast(z1n_all[0:128, 0:1], tc_ * 128,
                                   [[1024, 2], [1, 128]]),
                        in0=z1[:],
                        scalar1=st2[:, 0:1],
                        scalar2=rstd[:], op0=OP.subtract, op1=OP.mult)

                z1nTall = bigp.tile([128, 2 * L], BF, name="z1nTall", tag="z1nTall")
                nc.sync.dma_start_transpose(
                    out=_bcast(z1nTall[0:128, 0:1], 0, [[128, 16], [1, 128]]),
                    in_=z1n_all[:])

                # ---- MLP up (h.T orientation) + fused bias+relu -----------
                # h.T = W1eff.T-chunks @ z1nT; relu(x + b1) with b1 per-
                # partition in this orientation -> no r transpose needed.
                rTall = bigp.tile([128, 4 * L], BF, name="rTall", tag="rTall")
                for fc in range(4):
                    for hf in range(2):
                        ps = psM.tile([128, 512], F32, name="mm", tag="mm")
                        for dc in range(2):
                            nc.tensor.matmul(
                                ps[:],
                                lhsT=s_w1[:, dc * F + fc * 128:dc * F + (fc + 1) * 128],
                                rhs=z1nTall[:, dc * L + hf * 512:dc * L + (hf + 1) * 512],
                                start=(dc == 0), stop=(dc == 1))
                        dst = rTall[:, fc * L + hf * 512:fc * L + (hf + 1) * 512]
                        if (fc + hf) % 2 == 0:
                            nc.scalar.activation(out=dst, in_=ps[:], func=AF.Relu,
                                                 bias=s_b1col[:, fc:fc + 1])
                        else:
                            nc.vector.tensor_scalar(
                                out=dst, in0=ps[:], scalar1=s_b1col[:, fc:fc + 1],
                                scalar2=0.0, op0=OP.add, op1=OP.max)

                # ---- MLP down + injects + LN2 + out -----------------------
                ot_all = bigp.tile([128, 8 * D], F32, name="ot_all", tag="ot_all")
                for tc_ in range(8):
                    ps = psM.tile([128, D], F32, name="mm", tag="mm")
                    for fc in range(4):
                        nc.tensor.matmul(
                            ps[:], lhsT=rTall[:, fc * L + tc_ * 128:fc * L + (tc_ + 1) * 128],
                            rhs=s_w2[:, fc * D:(fc + 1) * D],
                            start=(fc == 0), stop=False)
                    for dc in range(2):
                        nc.tensor.matmul(
                            ps[:], lhsT=z1nTall[:, dc * L + tc_ * 128:dc * L + (tc_ + 1) * 128],
                            rhs=s_dg1[dc][:], start=False, stop=False)
                    nc.tensor.matmul(ps[:], lhsT=s_ones1[:], rhs=s_browz[:],
                                     start=False, stop=True)
                    st6 = smp.tile([128, 6], F32, name="st6", tag="st6")
                    st2 = smp.tile([128, 2], F32, name="st2", tag="st2")
                    nc.vector.bn_stats(st6[:], ps[:])
                    nc.vector.bn_aggr(st2[:], st6[:])
                    std = smp.tile([128, 1], F32, name="std", tag="std")
                    nc.scalar.activation(out=std[:], in_=st2[:, 1:2], func=AF.Sqrt,
                                         bias=s_eps[:, 0:1])
                    rstd = smp.tile([128, 1], F32, name="rstd", tag="rstd")
                    nc.vector.reciprocal(rstd[:], std[:])
                    t1 = smp.tile([128, D], F32, name="t1", tag="t1")
                    nc.vector.scalar_tensor_tensor(
                        out=t1[:], in0=ps[:], scalar=st2[:, 0:1], in1=s_g2b[:],
                        op0=OP.subtract, op1=OP.mult)
                    nc.vector.scalar_tensor_tensor(
                        out=ot_all[:, tc_ * D:(tc_ + 1) * D], in0=t1[:],
                        scalar=rstd[:], in1=s_b2gb[:],
                        op0=OP.mult, op1=OP.add)
                if s + 1 < SPC:
                    xpose_x(s + 1)
                nc.sync.dma_start(
                    out=_mkap(out[s, 0:1, 0:1], 0,
                              [[D, 128], [128 * D, 8], [1, D]]),
                    in_=ot_all[:])

    _split_sync_waits(nc)
    _CACHE["nc"] = nc
    return nc


def _in_maps(X, Wq, Wk, Wv, Wo, ln1_g, ln1_b, W1, b1, W2, b2, ln2_g, ln2_b):
    X = np.asarray(X, dtype=np.float32)
    f32 = lambda a: np.asarray(a, dtype=np.float32)
    Wq, Wk, Wv, Wo = f32(Wq), f32(Wk), f32(Wv), f32(Wo)
    W1, W2 = f32(W1), f32(W2)
    ln1_g, ln1_b, b1, b2 = f32(ln1_g), f32(ln1_b), f32(b1), f32(b2)
    ln2_g, ln2_b = f32(ln2_g), f32(ln2_b)

    bf = ml_dtypes.bfloat16
    w1eff = (ln1_g[:, None] * W1)
    b1eff = (b1 + ln1_b @ W1)
    dg1 = np.zeros((2, 128, D), np.float32)
    for c in range(2):
        for i in range(128):
            dg1[c, i, c * 128 + i] = ln1_g[c * 128 + i]
    m2d = _mask2d()
    mt_core = np.stack([m2d[0:128, 64:384], m2d[128:256, 192:512]])
    m01v = (mt_core == 0.0).astype(np.float32)  # [2, 128, 320] 0/1
    cm = _causal_mask()
    statics = {
        "wq": (Wq * (DH ** -0.5)).astype(bf),
        "wk": Wk.astype(bf),
        "wv": Wv.astype(bf),
        "wo": Wo.astype(bf),
        "w1": w1eff.astype(bf),
        "w2": W2.astype(bf),
        "dg1": dg1.astype(bf),
        "b1col": b1eff.reshape(4, 128).T.astype(np.float32).copy(),
        "browz": (ln1_b + b2)[None, :].astype(bf),
        "g2b": np.tile(ln2_g[None, :], (128, 1)).astype(np.float32),
        "b2gb": np.tile(ln2_b[None, :], (128, 1)).astype(np.float32),
        "ones1": np.ones((1, 128)).astype(bf),
        "m01": m01v.astype(bf),
        "m01b0": (cm[0:128, 0:128] == 0.0).astype(np.float32).astype(bf),
        "m01b1": (cm[128:256, 0:256] == 0.0).astype(np.float32).astype(bf),
    }

    in_maps = []
    for i in range(N_CORES):
        m = {"X": X[i * SPC:(i + 1) * SPC]}
        m.update(statics)
        in_maps.append(m)
    return in_maps


def kernel(**inputs):
    from concourse.bass_utils import run_bass_kernel_spmd
    nc = _build()
    res = run_bass_kernel_spmd(nc, _in_maps(**inputs), list(range(N_CORES)))
    return np.concatenate([res.results[i]["OUT"] for i in range(N_CORES)], axis=0)


def kernel_profiled(tmpdir=None, **inputs):
    from concourse.bass_utils import run_bass_kernel_spmd
    nc = _build()
    res = run_bass_kernel_spmd(nc, _in_maps(**inputs), list(range(N_CORES)),
                               trace=True, tmpdir=tmpdir)
    out = np.concatenate([res.results[i]["OUT"] for i in range(N_CORES)], axis=0)
    return out, res

